# revision 36
# baseline (speedup 1.0000x reference)
"""Trainium2 Bass kernel for a 2-layer GAT encoder (edge-softmax message passing).

Strategy (8 NeuronCores, SPMD single program):
- dst-node partition across cores; host packs each core's dst nodes into
  fixed-count "windows" (<=128 nodes each) and edges into fixed-count
  128-edge tiles per window (K_LO tiles for src in the low half of the
  padded node space, K_HI for the high half -- dma_gather indices are int16).
- Node phase: h_ext = x @ [W | W@att_src | W@att_dst] per core slice
  (node-major matmuls with x^T chunks stationary), fp16 row table written to
  DRAM, AllGather -> full table on every core.
- Edge phase per 128-edge tile: dma_gather rows by src; one-hot matrices
  built on DVE by iota-vs-dstrel compares (all operands packed fp16 so the
  DVE runs in 2x mode; the dst-slot column is pre-broadcast on the otherwise
  idle Act engine); per-edge a_d via one-hot matmul;
  p = exp(leaky_relu(a_s+a_d)) with the leaky-relu+exp on the Act engine
  (softmax shift invariance makes the segment-max subtraction unnecessary);
  segment-sum of [p*h | p] via one-hot matmul accumulated in PSUM per
  window; epilogue divides and applies bias/ELU (ELU tail fused via
  scalar_tensor_tensor).
- Node-phase matmuls run in fp16 (x and W1 shipped as fp16).
- SWDGE gather calls are batched at 1536 indices (48KB descriptor ring) to
  amortize the per-call Q7 launch overhead.
- Output rows are window-padded; host de-permutes to the original node order.
"""

import numpy as np

NCORES = 8
HALF = 32768          # int16 gather index limit
ROW1 = 256            # fp16 elems per layer-1 table row (512B): h128|a_s4|a_d4|pad
ROW2 = 128            # fp16 elems per layer-2 table row (256B): h64|a_s|a_d|pad
H1, C1 = 4, 32
H2, C2 = 1, 64
IN_CH = 128
HC1 = H1 * C1         # 128
NEG_SLOPE = 0.2
EPS = 1e-16


# ---------------------------------------------------------------------------
# Host-side preprocessing
# ---------------------------------------------------------------------------

def _pack_windows(src, dst, n_nodes, k_lo, k_hi, boundary):
    """Greedy-pack each core's dst nodes into windows (<=128 nodes, <=k_lo
    lo-tiles, <=k_hi hi-tiles). Returns per-core list of windows; each window
    is (node_lo, node_hi, lo_edge_srcs, hi_edge_srcs, lo_dstrel, hi_dstrel).
    Edges must be sorted by dst."""
    per_core = n_nodes // NCORES
    cores = []
    # bucket edges by dst once
    order = np.argsort(dst, kind="stable")
    src_s, dst_s = src[order], dst[order]
    # node -> edge range (dst-sorted)
    counts = np.bincount(dst_s, minlength=n_nodes)
    starts = np.concatenate([[0], np.cumsum(counts)])
    for c in range(NCORES):
        lo_n, hi_n = c * per_core, (c + 1) * per_core
        wins = []
        n = lo_n
        while n < hi_n:
            w_nodes = 0
            w_lo = []
            w_hi = []
            w_lo_dr = []
            w_hi_dr = []
            base = n
            while n < hi_n and w_nodes < 128:
                e0, e1 = starts[n], starts[n + 1]
                es = src_s[e0:e1]
                lo_m = es < boundary
                nlo = int(lo_m.sum())
                nhi = es.shape[0] - nlo
                cur_lo = sum(len(a) for a in w_lo)
                cur_hi = sum(len(a) for a in w_hi)
                if cur_lo + nlo > k_lo * 128 or cur_hi + nhi > k_hi * 128:
                    break
                w_lo.append(es[lo_m])
                w_hi.append(es[~lo_m])
                w_lo_dr.append(np.full(nlo, w_nodes, np.int32))
                w_hi_dr.append(np.full(nhi, w_nodes, np.int32))
                w_nodes += 1
                n += 1
            assert w_nodes > 0, "single node exceeds tile budget"
            wins.append((base, n,
                         np.concatenate(w_lo) if w_lo else np.zeros(0, src.dtype),
                         np.concatenate(w_hi) if w_hi else np.zeros(0, src.dtype),
                         np.concatenate(w_lo_dr) if w_lo_dr else np.zeros(0, np.int32),
                         np.concatenate(w_hi_dr) if w_hi_dr else np.zeros(0, np.int32)))
        cores.append(wins)
    return cores


def host_prep(x, edge_index, n_nodes, k_lo, k_hi, gw):
    """Build the permutation, per-core metadata and index arrays."""
    src = np.ascontiguousarray(edge_index[0]).astype(np.int64)
    dst = np.ascontiguousarray(edge_index[1]).astype(np.int64)
    per_core = n_nodes // NCORES
    assert per_core * NCORES == n_nodes

    # fixpoint on the lo/hi boundary in *real node id* space (pi is monotone)
    boundary = min(n_nodes, HALF)
    for _ in range(6):
        cores = _pack_windows(src, dst, n_nodes, k_lo, k_hi, boundary)
        W = max(len(w) for w in cores)
        W = ((W + gw - 1) // gw) * gw  # pad to group multiple
        P = W * 128
        # pi: node -> padded slot id
        pi = np.zeros(n_nodes, np.int64)
        for c, wins in enumerate(cores):
            for w, (a, b, *_r) in enumerate(wins):
                ids = np.arange(a, b)
                pi[ids] = c * P + w * 128 + (ids - a)
        nb = int(np.searchsorted(pi, HALF))  # first node with pi >= HALF
        if nb == boundary or P * NCORES <= HALF:
            boundary = nb if P * NCORES > HALF else n_nodes
            break
        boundary = nb
    cores = _pack_windows(src, dst, n_nodes, k_lo, k_hi, boundary)
    W = max(len(w) for w in cores)
    W = ((W + gw - 1) // gw) * gw
    P = W * 128
    pi = np.zeros(n_nodes, np.int64)
    for c, wins in enumerate(cores):
        for w, (a, b, *_r) in enumerate(wins):
            ids = np.arange(a, b)
            pi[ids] = c * P + w * 128 + (ids - a)
    assert P * NCORES <= 65536, f"padded node space {P*NCORES} exceeds uint16 gather range"
    hi_exists = P * NCORES > HALF
    if not hi_exists:
        assert all(len(w[3]) == 0 for ws in cores for w in ws)

    K = k_lo + k_hi
    meta = []
    for c, wins in enumerate(cores):
        idx_lo = np.zeros((W, k_lo * 128), np.int16)
        idx_hi = np.zeros((W, k_hi * 128), np.int16)
        drel = np.full((W, K, 128), 255, np.int32)  # [window, tile-in-window, slot]
        for w, (a, b, lo_s, hi_s, lo_dr, hi_dr) in enumerate(wins):
            pl = pi[lo_s]
            assert (pl < HALF).all()
            idx_lo[w, :len(pl)] = pl.astype(np.int16)
            # lo tiles occupy tile slots [0, k_lo)
            dr_pad = np.full(k_lo * 128, 255, np.int32)
            dr_pad[:len(lo_dr)] = lo_dr
            drel[w, :k_lo] = dr_pad.reshape(k_lo, 128)
            if hi_exists:
                ph = pi[hi_s] - HALF
                assert (ph >= 0).all() and (ph < 32768).all()
                idx_hi[w, :len(ph)] = ph.astype(np.int16)
            dr_pad = np.full(k_hi * 128, 255, np.int32)
            dr_pad[:len(hi_dr)] = hi_dr
            drel[w, k_lo:] = dr_pad.reshape(k_hi, 128) if k_hi else drel[w, k_lo:]
        # group-tile order: per group: [lo tiles of gw windows][hi tiles of gw windows]
        G = W // gw
        tile_order = []  # (window, tile-in-window-index)
        for g in range(G):
            for w in range(g * gw, (g + 1) * gw):
                tile_order += [(w, t) for t in range(k_lo)]
            for w in range(g * gw, (g + 1) * gw):
                tile_order += [(w, k_lo + t) for t in range(k_hi)]
        to = np.array(tile_order)
        drel_t = drel[to[:, 0], to[:, 1]]            # [W*K, 128]
        # idx arrays in 16-partition wrapped layout: idx j -> [j%16, j//16]
        def wrap16(a):
            a = a.reshape(-1)
            # idx j lives at [j%16, j//16], replicated into all 8 Q7 core
            # partition groups (HW reads each group independently)
            return np.ascontiguousarray(np.tile(a.reshape(-1, 16).T, (8, 1)))
        meta.append(dict(
            idx_lo=wrap16(idx_lo),
            idx_hi=wrap16(idx_hi) if hi_exists else None,
            dc=np.ascontiguousarray(drel_t.T).astype(np.float16),   # [128, W*K]
            dr=np.ascontiguousarray(drel_t).astype(np.float16),     # [W*K, 128]
        ))
    return dict(cores=cores, pi=pi, W=W, P=P, K=K, k_lo=k_lo, k_hi=k_hi, gw=gw,
                hi_exists=hi_exists, meta=meta, n_nodes=n_nodes, per_core=per_core)


def pick_config(x, edge_index, n_nodes):
    """Try candidate (k_lo, k_hi) packings, return the prep with fewest tiles."""
    gw = 2
    E = edge_index.shape[1]
    lam = E / n_nodes * 128
    base_lo = max(int(np.ceil(lam * 0.64 / 128)), 1)
    base_hi = max(int(np.ceil(lam * 0.36 / 128)), 0)
    cands = []
    for dlo in (-1, 0, 1, 2):
        for dhi in (-1, 0, 1, 2):
            if base_lo + dlo >= 1 and base_hi + dhi >= 0:
                cands.append((base_lo + dlo, base_hi + dhi))
    cands.sort(key=lambda c: c[0] + c[1])
    best = None
    for k_lo, k_hi in cands:
        try:
            p = host_prep(x, edge_index, n_nodes, k_lo, k_hi, gw)
        except AssertionError:
            continue
        slots = p["W"] * p["K"]
        if best is None or slots < best["W"] * best["K"]:
            best = p
        if slots <= (E / NCORES) / 128 * 1.08:  # good enough
            break
    assert best is not None, "no feasible packing found"
    return best


# ---------------------------------------------------------------------------
# Bass program
# ---------------------------------------------------------------------------

def build_program(cfg):
    import os
    KSTOP = int(os.environ.get("KSTOP", "0"))  # debug: 1=phaseA 2=+AG1 3=+edge1 4=+phaseC+AG2
    import concourse.bacc as bacc
    import concourse.bass as bass
    import concourse.mybir as mybir
    from concourse import tile

    f32 = mybir.dt.float32
    f16 = mybir.dt.float16
    i16 = mybir.dt.int16
    AF = mybir.ActivationFunctionType
    OP = mybir.AluOpType

    W, P, K, k_lo, k_hi, gw = cfg["W"], cfg["P"], cfg["K"], cfg["k_lo"], cfg["k_hi"], cfg["gw"]
    hi_exists = cfg["hi_exists"]
    G = W // gw
    T = gw * K                  # tiles per group
    GCAP = 1024                 # max gather descriptors per SWDGE call
    NLO = gw * k_lo * 128       # lo gather idxs per group
    NHI = gw * k_hi * 128
    P_ALL = P * NCORES

    nc = bacc.Bacc("TRN2", target_bir_lowering=False, debug=False, num_devices=NCORES,
                   dynamic_dma_scratch_size=49152)

    # ---- external inputs ----
    xT = nc.dram_tensor("xT", [IN_CH, P], f16, kind="ExternalInput")
    w1e = nc.dram_tensor("w1e", [IN_CH, 136], f16, kind="ExternalInput")
    w2e = nc.dram_tensor("w2e", [HC1, 66], f32, kind="ExternalInput")
    idx_lo_d = nc.dram_tensor("idx_lo", [128, W * k_lo * 8], i16, kind="ExternalInput")
    if hi_exists:
        idx_hi_d = nc.dram_tensor("idx_hi", [128, W * k_hi * 8], i16, kind="ExternalInput")
    u8 = mybir.dt.uint8
    dc_d = nc.dram_tensor("dc", [128, W * K], f16, kind="ExternalInput")
    dr_d = nc.dram_tensor("dr", [W * K, 128], f16, kind="ExternalInput")
    iota_c_d = nc.dram_tensor("iota_c", [128, 1], f16, kind="ExternalInput")
    iota_r_d = nc.dram_tensor("iota_r", [128, 128], f16, kind="ExternalInput")
    ident_d = nc.dram_tensor("ident", [128, 128], f16, kind="ExternalInput")
    b1_d = nc.dram_tensor("b1", [128, HC1], f32, kind="ExternalInput")
    b2_d = nc.dram_tensor("b2", [128, C2], f32, kind="ExternalInput")
    out_d = nc.dram_tensor("out", [P, C2], f32, kind="ExternalOutput")

    with tile.TileContext(nc) as tc:
        with (
            tc.tile_pool(name="const", bufs=1) as cpool,
            tc.tile_pool(name="xc", bufs=3) as xcpool,
            tc.tile_pool(name="rows", bufs=3) as rowpool,
            tc.tile_pool(name="gather", bufs=2) as gpool,
            tc.tile_pool(name="onehot", bufs=2) as opool,
            tc.tile_pool(name="rmat", bufs=2) as rpool,
            tc.tile_pool(name="scal", bufs=3) as spool,
            tc.tile_pool(name="idx", bufs=3) as ipool,
            tc.tile_pool(name="epi", bufs=3) as epool,
            tc.tile_pool(name="psA", bufs=2, space="PSUM") as psA,
            tc.tile_pool(name="psW", bufs=4, space="PSUM") as psW,
            tc.tile_pool(name="psad", bufs=2, space="PSUM") as psad,
            tc.tile_pool(name="dram", bufs=1, space="DRAM") as dpool,
        ):
            # ---- constants to SBUF ----
            w1e_sb = cpool.tile([IN_CH, 136], f16, tag="w1e")
            nc.sync.dma_start(w1e_sb[:], w1e[:])
            w2e_sb = cpool.tile([HC1, 66], f32, tag="w2e")
            nc.sync.dma_start(w2e_sb[:], w2e[:])
            w2e_f16 = cpool.tile([HC1, 66], f16, tag="w2e16")
            nc.vector.tensor_copy(w2e_f16[:], w2e_sb[:])
            iota_c = cpool.tile([128, 1], f16, tag="iota_c")
            nc.sync.dma_start(iota_c[:], iota_c_d[:])
            iota_r = cpool.tile([128, 128], f16, tag="iota_r")
            nc.sync.dma_start(iota_r[:], iota_r_d[:])
            # iota_part[p, :] == p, packed f16 (keeps the sm build in 2x DVE mode)
            iota_part = cpool.tile([128, T * 128], f16, tag="iota_part")
            nc.scalar.activation(iota_part[:], iota_c[:].broadcast_to([128, T * 128]),
                                 AF.Copy)
            ident = cpool.tile([128, 128], f16, tag="ident")
            nc.sync.dma_start(ident[:], ident_d[:])
            b1_sb = cpool.tile([128, HC1], f32, tag="b1")
            nc.sync.dma_start(b1_sb[:], b1_d[:])
            b2_sb = cpool.tile([128, C2], f32, tag="b2")
            nc.sync.dma_start(b2_sb[:], b2_d[:])
            adsb1 = cpool.tile([128, W, H1], f16, tag="adsb1")
            adsb2 = cpool.tile([128, W, H2], f16, tag="adsb2")
            x2T = cpool.tile([128, P], f16, tag="x2T")

            # ---- DRAM tables ----
            tab1_slice = dpool.tile([P, ROW1], f16, tag="t1s")
            tab1_full = dpool.tile([P_ALL, ROW1], f16, tag="t1f", addr_space="Shared")
            tab2_slice = dpool.tile([P, ROW2], f16, tag="t2s")
            tab2_full = dpool.tile([P_ALL, ROW2], f16, tag="t2f", addr_space="Shared")

            # ================= phase A: layer-1 node matmul =================
            import os as _os2
            _kpha = _os2.environ.get("KPHA") == "1"  # debug: no matmuls in phase A
            for w in range(W):
                xc = xcpool.tile([IN_CH, 128], f16, tag="xc")
                nc.sync.dma_start(xc[:], xT[:, w * 128:(w + 1) * 128])
                rows = rowpool.tile([128, ROW1], f16, tag="rows1")
                if _kpha:
                    nc.vector.memset(rows[:], 0.25)
                    nc.vector.memset(adsb1[:, w, :], 0.25)
                else:
                    ps = psA.tile([128, 136], f32, tag="ps_node")
                    nc.tensor.matmul(ps[:], lhsT=xc[:], rhs=w1e_sb[:], start=True, stop=True)
                    nc.vector.memset(rows[:, 136:], 0.0)
                    nc.vector.tensor_copy(rows[:, 0:136], ps[:])
                    nc.vector.tensor_copy(adsb1[:, w, :], ps[:, 132:136])
                nc.sync.dma_start(tab1_slice[w * 128:(w + 1) * 128, :], rows[:])
            if KSTOP != 1:
                nc.gpsimd.collective_compute(
                    "AllGather", mybir.AluOpType.bypass,
                    replica_groups=[list(range(NCORES))],
                    ins=[tab1_slice.opt()], outs=[tab1_full.opt()],
                )

            # ================= edge phase (shared for both layers) ==========
            def edge_phase(layer):
                import os as _os
                _ked = int(_os.environ.get("KEDGE", "9"))
                _nog = _os.environ.get("KNOGATHER") == "1"

                if layer == 1:
                    table, row, heads, ch = tab1_full, ROW1, H1, C1
                    adsb = adsb1
                    a_s_off, a_d_off = 128, 132
                    rcols = HC1 + H1  # 132
                else:
                    table, row, heads, ch = tab2_full, ROW2, H2, C2
                    adsb = adsb2
                    a_s_off, a_d_off = 64, 65
                    rcols = C2 + H2  # 65
                hc = heads * ch
                nh = heads  # scalar cols per tile
                for g in range(G):
                    w0 = g * gw
                    # --- gather ---
                    Gt = gpool.tile([128, T, row], f16, tag=f"G{layer}")
                    il = ipool.tile([128, NLO // 16], i16, tag="il")
                    nc.sync.dma_start(il[:], idx_lo_d[:, g * (NLO // 16):(g + 1) * (NLO // 16)])
                    if not _nog:
                        for off in range(0, NLO, GCAP):
                            sz = min(GCAP, NLO - off)
                            nc.gpsimd.dma_gather(
                                out_ap=Gt[:, off // 128:(off + sz) // 128, :],
                                in_ap=table[0:min(HALF, P_ALL), :],
                                idxs_ap=il[:, off // 16:(off + sz) // 16],
                                num_idxs=sz, num_idxs_reg=sz,
                                elem_size=row)
                    else:
                        nc.vector.memset(Gt[:].rearrange("p t r -> p (t r)"), 0.25)
                    if hi_exists and k_hi > 0:
                        ih = ipool.tile([128, NHI // 16], i16, tag="ih")
                        nc.sync.dma_start(ih[:], idx_hi_d[:, g * (NHI // 16):(g + 1) * (NHI // 16)])
                        if not _nog:
                            for off in range(0, NHI, GCAP):
                                sz = min(GCAP, NHI - off)
                                nc.gpsimd.dma_gather(
                                    out_ap=Gt[:, gw * k_lo + off // 128:gw * k_lo + (off + sz) // 128, :],
                                    in_ap=table[HALF:P_ALL, :],
                                    idxs_ap=ih[:, off // 16:(off + sz) // 16],
                                    num_idxs=sz, num_idxs_reg=sz,
                                    elem_size=row)
                    # --- one-hot builds ---
                    if _ked < 2:
                        dmy = spool.tile([128, T, row], f16, tag="dmyG")
                        nc.vector.tensor_copy(dmy[:], Gt[:])
                        continue
                    dc_sb = ipool.tile([128, T], f16, tag="dc")
                    nc.sync.dma_start(dc_sb[:], dc_d[:, g * T:(g + 1) * T])
                    dr_rep = ipool.tile([128, T * 128], f16, tag="dr_rep")
                    nc.sync.dma_start(
                        dr_rep[:],
                        dr_d[g * T:(g + 1) * T, :]
                        .rearrange("(one a) b -> one (a b)", one=1)
                        .partition_broadcast(128).opt())
                    # expand dc on the (idle) Act engine so the DVE compare
                    # sees packed 2-byte operands and runs in 2x mode
                    dc_exp = opool.tile([128, T, 128], f16, tag="dc_exp")
                    nc.scalar.activation(
                        dc_exp[:],
                        dc_sb[:].rearrange("p (t one) -> p t one", one=1).broadcast_to([128, T, 128]),
                        AF.Copy)
                    e1 = opool.tile([128, T, 128], f16, tag="e1")
                    nc.vector.tensor_tensor(
                        e1[:],
                        dc_exp[:],
                        iota_r[:].rearrange("p (one x) -> p one x", one=1).broadcast_to([128, T, 128]),
                        OP.is_equal)
                    sm = opool.tile([128, T, 128], f16, tag="sm")
                    nc.vector.tensor_tensor(
                        sm[:].rearrange("p t j -> p (t j)"),
                        iota_part[:],
                        dr_rep[:],
                        OP.is_equal)
                    if _ked < 3:
                        continue
                    # --- a_d broadcast matmuls ---
                    pad = psad.tile([128, T * nh], f32, tag="pad")
                    for t in range(T):
                        w = w0 + (t // k_lo if t < gw * k_lo else (t - gw * k_lo) // k_hi)
                        nc.tensor.matmul(pad[:, t * nh:(t + 1) * nh],
                                         lhsT=sm[:, t, :], rhs=adsb[:, w, :],
                                         start=True, stop=True)
                    if _ked < 4:
                        continue
                    # --- per-edge scalars ---
                    z = spool.tile([128, T, nh], f32, tag="z")
                    nc.vector.tensor_add(z[:], Gt[:, :, a_s_off:a_s_off + nh],
                                         pad[:].rearrange("p (t h) -> p t h", h=nh))
                    z2 = spool.tile([128, T, nh], f32, tag="z2")
                    nc.vector.tensor_scalar_mul(z2[:], z[:], NEG_SLOPE)
                    nc.vector.tensor_max(z[:], z[:], z2[:])
                    psc = spool.tile([128, T, nh], f16, tag="psc")
                    nc.scalar.activation(psc[:], z[:], AF.Exp)
                    if _ked < 5:
                        continue
                    # --- R = [p*h | p] ---
                    # expand psc per-head on the Act engine so the DVE mul is
                    # a single packed-f16 2x-mode op
                    R = rpool.tile([128, T, rcols], f16, tag=f"R")
                    for h in range(heads):
                        nc.vector.tensor_mul(
                            R[:, :, h * ch:(h + 1) * ch],
                            Gt[:, :, h * ch:(h + 1) * ch],
                            psc[:, :, h:h + 1].broadcast_to([128, T, ch]))
                    nc.vector.tensor_copy(R[:, :, hc:hc + nh], psc[:])
                    nc.vector.tensor_copy(R[:, :, hc:hc + nh], psc[:])
                    if _ked < 6:
                        continue
                    # --- segment-sum matmuls ---
                    pw = [psW.tile([128, rcols], f32, tag="psW", name=f"pw{g}_{wi}")
                          for wi in range(gw)]
                    for t in range(T):
                        if t < gw * k_lo:
                            wi, first = divmod(t, k_lo)
                            is_first = first == 0
                            is_last = (first == k_lo - 1) and k_hi == 0
                        else:
                            wi, r = divmod(t - gw * k_lo, k_hi)
                            is_first = False
                            is_last = r == k_hi - 1
                        nc.tensor.matmul(pw[wi][:], lhsT=e1[:, t, :], rhs=R[:, t, :],
                                         start=is_first, stop=is_last)
                    # --- epilogue per window ---
                    import os as _os
                    _epi = int(_os.environ.get("KEPI", "0"))  # 6=no epilogue, 7=no transpose
                    for wi in range(gw):
                        if _epi == 6:
                            dummy = epool.tile([128, rcols], f16, tag="dummy")
                            nc.vector.tensor_copy(dummy[:], pw[wi][:])
                            continue
                        w = w0 + wi
                        den = epool.tile([128, nh], f32, tag="den")
                        nc.vector.tensor_scalar_add(den[:], pw[wi][:, hc:hc + nh], EPS)
                        rec = epool.tile([128, nh], f32, tag="rec")
                        nc.vector.reciprocal(rec[:], den[:])
                        o = epool.tile([128, hc], f32, tag="o")
                        nc.vector.tensor_mul(
                            o[:].rearrange("p (h c) -> p h c", h=heads),
                            pw[wi][:, 0:hc].rearrange("p (h c) -> p h c", h=heads),
                            rec[:].broadcast_to([128, heads, ch]))
                        if layer == 1:
                            nc.vector.tensor_add(o[:], o[:], b1_sb[:])
                            neg = epool.tile([128, hc], f32, tag="neg")
                            nc.vector.tensor_scalar_min(neg[:], o[:], 0.0)
                            nc.scalar.activation(neg[:], neg[:], AF.Exp)
                            pos = epool.tile([128, hc], f32, tag="pos")
                            nc.vector.tensor_scalar_max(pos[:], o[:], 0.0)
                            act = epool.tile([128, hc], f16, tag="act")
                            # act = (exp(neg) - 1) + pos  (ELU), fused on DVE
                            nc.vector.scalar_tensor_tensor(
                                act[:], neg[:], -1.0, pos[:],
                                OP.add, OP.add)
                            if _epi == 7:
                                nc.vector.tensor_copy(x2T[:, w * 128:(w + 1) * 128], act[:])
                            else:
                                psT = psA.tile([128, 128], f16, tag="ps_node")
                                nc.tensor.transpose(psT[:], act[:], ident[:])
                                nc.vector.tensor_copy(x2T[:, w * 128:(w + 1) * 128], psT[:])
                        else:
                            nc.vector.tensor_add(o[:], o[:], b2_sb[:])
                            nc.sync.dma_start(out_d[w * 128:(w + 1) * 128, :], o[:])

            if KSTOP in (0, 3, 4):
                edge_phase(1)

            # ================= phase C: layer-2 node matmul =================
            for w in range(W if KSTOP in (0, 4) else 0):
                ps = psA.tile([128, 66], f32, tag="ps_node")
                nc.tensor.matmul(ps[:], lhsT=x2T[:, w * 128:(w + 1) * 128],
                                 rhs=w2e_f16[:], start=True, stop=True)
                rows = rowpool.tile([128, ROW2], f16, tag="rows2")
                nc.vector.memset(rows[:, 66:], 0.0)
                nc.vector.tensor_copy(rows[:, 0:66], ps[:])
                nc.vector.tensor_copy(adsb2[:, w, :], ps[:, 65:66])
                nc.sync.dma_start(tab2_slice[w * 128:(w + 1) * 128, :], rows[:])
            if KSTOP in (0, 4):
                nc.gpsimd.collective_compute(
                    "AllGather", mybir.AluOpType.bypass,
                    replica_groups=[list(range(NCORES))],
                    ins=[tab2_slice.opt()], outs=[tab2_full.opt()],
                )

            if KSTOP == 0:
                edge_phase(2)

    nc.compile()
    return nc


# ---------------------------------------------------------------------------
# Entry point
# ---------------------------------------------------------------------------

_CACHE = {}


def _prepare(inputs):
    x = np.ascontiguousarray(np.asarray(inputs["x"], np.float32))
    ei = np.asarray(inputs["edge_index"])
    n_nodes = x.shape[0]
    return pick_config(x, ei, n_nodes)


def _weights_ext(inputs):
    W1 = np.asarray(inputs["W1"], np.float32)
    as1 = np.asarray(inputs["att_src1"], np.float32)
    ad1 = np.asarray(inputs["att_dst1"], np.float32)
    W2 = np.asarray(inputs["W2"], np.float32)
    as2 = np.asarray(inputs["att_src2"], np.float32)
    ad2 = np.asarray(inputs["att_dst2"], np.float32)
    As = np.zeros((HC1, H1), np.float32)
    Ad = np.zeros((HC1, H1), np.float32)
    for h in range(H1):
        As[h * C1:(h + 1) * C1, h] = as1[0, h]
        Ad[h * C1:(h + 1) * C1, h] = ad1[0, h]
    w1e = np.concatenate([W1, W1 @ As, W1 @ Ad], axis=1)           # [128,136]
    w2e = np.concatenate([W2, W2 @ as2[0].T, W2 @ ad2[0].T], axis=1)  # [128,66]
    return np.ascontiguousarray(w1e).astype(np.float16), np.ascontiguousarray(w2e)


def kernel(**inputs):
    from concourse.bass_utils import run_bass_kernel_spmd

    prep = _prepare(inputs)
    key = (prep["W"], prep["K"], prep["k_lo"], prep["k_hi"], prep["gw"], prep["hi_exists"])
    if key not in _CACHE:
        _CACHE[key] = build_program(dict(
            W=prep["W"], P=prep["P"], K=prep["K"], k_lo=prep["k_lo"],
            k_hi=prep["k_hi"], gw=prep["gw"], hi_exists=prep["hi_exists"]))
    nc = _CACHE[key]

    in_maps = build_in_maps(inputs, prep)
    res = run_bass_kernel_spmd(nc, in_maps, core_ids=list(range(NCORES)))
    return assemble_output(res.results, prep)


def build_in_maps(inputs, prep):
    x = np.ascontiguousarray(np.asarray(inputs["x"], np.float32))
    b1 = np.tile(np.asarray(inputs["b1"], np.float32).reshape(1, HC1), (128, 1))
    b2 = np.tile(np.asarray(inputs["b2"], np.float32).reshape(1, C2), (128, 1))
    w1e, w2e = _weights_ext(inputs)
    n_nodes, P, W = prep["n_nodes"], prep["P"], prep["W"]
    pi = prep["pi"]
    iota_c = np.arange(128, dtype=np.float16).reshape(128, 1)
    iota_r = np.tile(np.arange(128, dtype=np.float16), (128, 1))
    ident = np.eye(128, dtype=np.float16)
    in_maps = []
    # xT_pad per core: columns = padded slots
    xT_all = np.zeros((NCORES, IN_CH, P), np.float16)
    node_ids = np.arange(n_nodes)
    c_of = pi // P
    col = pi % P
    xT_all[c_of, :, col] = x[node_ids]  # fancy: sets [ch] vectors
    for c in range(NCORES):
        m = prep["meta"][c]
        im = dict(
            xT=np.ascontiguousarray(xT_all[c]),
            w1e=w1e, w2e=w2e,
            idx_lo=np.ascontiguousarray(m["idx_lo"]),
            dc=np.ascontiguousarray(m["dc"]),
            dr=np.ascontiguousarray(m["dr"]),
            iota_c=iota_c, iota_r=iota_r, ident=ident,
            b1=b1, b2=b2,
        )
        if prep["hi_exists"]:
            im["idx_hi"] = np.ascontiguousarray(m["idx_hi"])
        in_maps.append(im)
    return in_maps


def assemble_output(results, prep):
    P, n_nodes = prep["P"], prep["n_nodes"]
    full = np.concatenate([results[c]["out"] for c in range(NCORES)], axis=0)
    return np.ascontiguousarray(full[prep["pi"]]).astype(np.float32)



# revision 41
# speedup vs baseline: 1.0312x; 1.0312x over previous
"""Trainium2 Bass kernel for a 2-layer GAT encoder (edge-softmax message passing).

Strategy (8 NeuronCores, SPMD single program):
- dst-node partition across cores; host packs each core's dst nodes into
  fixed-count "windows" (<=128 nodes each) and edges into fixed-count
  128-edge tiles per window (K_LO tiles for src in the low half of the
  padded node space, K_HI for the high half -- dma_gather indices are int16).
- Node phase: h_ext = x @ [W | W@att_src | W@att_dst] per core slice
  (node-major matmuls with x^T chunks stationary), fp16 row table written to
  DRAM, AllGather -> full table on every core.
- Edge phase per 128-edge tile: dma_gather rows by src; one-hot matrices
  built on DVE by iota-vs-dstrel compares (all operands packed fp16 so the
  DVE runs in 2x mode; the dst-slot column is pre-broadcast on the otherwise
  idle Act engine); per-edge a_d via one-hot matmul;
  p = exp(leaky_relu(a_s+a_d)) with the leaky-relu+exp on the Act engine
  (softmax shift invariance makes the segment-max subtraction unnecessary);
  segment-sum of [p*h | p] via one-hot matmul accumulated in PSUM per
  window; epilogue divides and applies bias/ELU (ELU tail fused via
  scalar_tensor_tensor).
- Node-phase matmuls run in fp16 (x and W1 shipped as fp16).
- SWDGE gather calls are batched at 1536 indices (48KB descriptor ring) to
  amortize the per-call Q7 launch overhead.
- Output rows are window-padded; host de-permutes to the original node order.
"""

import numpy as np

NCORES = 8
HALF = 32768          # int16 gather index limit
ROW1 = 256            # fp16 elems per layer-1 table row (512B): h128|a_s4|a_d4|pad
ROW2 = 128            # fp16 elems per layer-2 table row (256B): h64|a_s|a_d|pad
H1, C1 = 4, 32
H2, C2 = 1, 64
IN_CH = 128
HC1 = H1 * C1         # 128
NEG_SLOPE = 0.2
EPS = 1e-16


# ---------------------------------------------------------------------------
# Host-side preprocessing
# ---------------------------------------------------------------------------

def _pack_windows(src, dst, n_nodes, k_lo, k_hi, boundary):
    """Greedy-pack each core's dst nodes into windows (<=128 nodes, <=k_lo
    lo-tiles, <=k_hi hi-tiles). Returns per-core list of windows; each window
    is (node_lo, node_hi, lo_edge_srcs, hi_edge_srcs, lo_dstrel, hi_dstrel).
    Edges must be sorted by dst."""
    per_core = n_nodes // NCORES
    cores = []
    # bucket edges by dst once
    order = np.argsort(dst, kind="stable")
    src_s, dst_s = src[order], dst[order]
    # node -> edge range (dst-sorted)
    counts = np.bincount(dst_s, minlength=n_nodes)
    starts = np.concatenate([[0], np.cumsum(counts)])
    for c in range(NCORES):
        lo_n, hi_n = c * per_core, (c + 1) * per_core
        wins = []
        n = lo_n
        while n < hi_n:
            w_nodes = 0
            w_lo = []
            w_hi = []
            w_lo_dr = []
            w_hi_dr = []
            base = n
            while n < hi_n and w_nodes < 128:
                e0, e1 = starts[n], starts[n + 1]
                es = src_s[e0:e1]
                lo_m = es < boundary
                nlo = int(lo_m.sum())
                nhi = es.shape[0] - nlo
                cur_lo = sum(len(a) for a in w_lo)
                cur_hi = sum(len(a) for a in w_hi)
                if cur_lo + nlo > k_lo * 128 or cur_hi + nhi > k_hi * 128:
                    break
                w_lo.append(es[lo_m])
                w_hi.append(es[~lo_m])
                w_lo_dr.append(np.full(nlo, w_nodes, np.int32))
                w_hi_dr.append(np.full(nhi, w_nodes, np.int32))
                w_nodes += 1
                n += 1
            assert w_nodes > 0, "single node exceeds tile budget"
            wins.append((base, n,
                         np.concatenate(w_lo) if w_lo else np.zeros(0, src.dtype),
                         np.concatenate(w_hi) if w_hi else np.zeros(0, src.dtype),
                         np.concatenate(w_lo_dr) if w_lo_dr else np.zeros(0, np.int32),
                         np.concatenate(w_hi_dr) if w_hi_dr else np.zeros(0, np.int32)))
        cores.append(wins)
    return cores


def host_prep(x, edge_index, n_nodes, k_lo, k_hi, gw):
    """Build the permutation, per-core metadata and index arrays."""
    src = np.ascontiguousarray(edge_index[0]).astype(np.int64)
    dst = np.ascontiguousarray(edge_index[1]).astype(np.int64)
    per_core = n_nodes // NCORES
    assert per_core * NCORES == n_nodes

    # fixpoint on the lo/hi boundary in *real node id* space (pi is monotone)
    boundary = min(n_nodes, HALF)
    for _ in range(6):
        cores = _pack_windows(src, dst, n_nodes, k_lo, k_hi, boundary)
        W = max(len(w) for w in cores)
        W = ((W + gw - 1) // gw) * gw  # pad to group multiple
        P = W * 128
        # pi: node -> padded slot id
        pi = np.zeros(n_nodes, np.int64)
        for c, wins in enumerate(cores):
            for w, (a, b, *_r) in enumerate(wins):
                ids = np.arange(a, b)
                pi[ids] = c * P + w * 128 + (ids - a)
        nb = int(np.searchsorted(pi, HALF))  # first node with pi >= HALF
        if nb == boundary or P * NCORES <= HALF:
            boundary = nb if P * NCORES > HALF else n_nodes
            break
        boundary = nb
    cores = _pack_windows(src, dst, n_nodes, k_lo, k_hi, boundary)
    W = max(len(w) for w in cores)
    W = ((W + gw - 1) // gw) * gw
    P = W * 128
    pi = np.zeros(n_nodes, np.int64)
    for c, wins in enumerate(cores):
        for w, (a, b, *_r) in enumerate(wins):
            ids = np.arange(a, b)
            pi[ids] = c * P + w * 128 + (ids - a)
    assert P * NCORES <= 65536, f"padded node space {P*NCORES} exceeds uint16 gather range"
    hi_exists = P * NCORES > HALF
    if not hi_exists:
        assert all(len(w[3]) == 0 for ws in cores for w in ws)

    K = k_lo + k_hi
    meta = []
    for c, wins in enumerate(cores):
        idx_lo = np.zeros((W, k_lo * 128), np.int16)
        idx_hi = np.zeros((W, k_hi * 128), np.int16)
        drel = np.full((W, K, 128), 255, np.int32)  # [window, tile-in-window, slot]
        for w, (a, b, lo_s, hi_s, lo_dr, hi_dr) in enumerate(wins):
            pl = pi[lo_s]
            assert (pl < HALF).all()
            idx_lo[w, :len(pl)] = pl.astype(np.int16)
            # lo tiles occupy tile slots [0, k_lo)
            dr_pad = np.full(k_lo * 128, 255, np.int32)
            dr_pad[:len(lo_dr)] = lo_dr
            drel[w, :k_lo] = dr_pad.reshape(k_lo, 128)
            if hi_exists:
                ph = pi[hi_s] - HALF
                assert (ph >= 0).all() and (ph < 32768).all()
                idx_hi[w, :len(ph)] = ph.astype(np.int16)
            dr_pad = np.full(k_hi * 128, 255, np.int32)
            dr_pad[:len(hi_dr)] = hi_dr
            drel[w, k_lo:] = dr_pad.reshape(k_hi, 128) if k_hi else drel[w, k_lo:]
        # group-tile order: per group: [lo tiles of gw windows][hi tiles of gw windows]
        G = W // gw
        tile_order = []  # (window, tile-in-window-index)
        for g in range(G):
            for w in range(g * gw, (g + 1) * gw):
                tile_order += [(w, t) for t in range(k_lo)]
            for w in range(g * gw, (g + 1) * gw):
                tile_order += [(w, k_lo + t) for t in range(k_hi)]
        to = np.array(tile_order)
        drel_t = drel[to[:, 0], to[:, 1]]            # [W*K, 128]
        # idx arrays in 16-partition wrapped layout: idx j -> [j%16, j//16]
        def wrap16(a):
            a = a.reshape(-1)
            # idx j lives at [j%16, j//16], replicated into all 8 Q7 core
            # partition groups (HW reads each group independently)
            return np.ascontiguousarray(np.tile(a.reshape(-1, 16).T, (8, 1)))
        meta.append(dict(
            idx_lo=wrap16(idx_lo),
            idx_hi=wrap16(idx_hi) if hi_exists else None,
            dc=np.ascontiguousarray(drel_t.T).astype(np.float16),   # [128, W*K]
            dr=np.ascontiguousarray(drel_t).astype(np.float16),     # [W*K, 128]
        ))
    return dict(cores=cores, pi=pi, W=W, P=P, K=K, k_lo=k_lo, k_hi=k_hi, gw=gw,
                hi_exists=hi_exists, meta=meta, n_nodes=n_nodes, per_core=per_core)


def pick_config(x, edge_index, n_nodes):
    """Try candidate (k_lo, k_hi) packings, return the prep with fewest tiles."""
    gw = 2
    E = edge_index.shape[1]
    lam = E / n_nodes * 128
    base_lo = max(int(np.ceil(lam * 0.64 / 128)), 1)
    base_hi = max(int(np.ceil(lam * 0.36 / 128)), 0)
    cands = []
    for dlo in (-1, 0, 1, 2):
        for dhi in (-1, 0, 1, 2):
            if base_lo + dlo >= 1 and base_hi + dhi >= 0:
                cands.append((base_lo + dlo, base_hi + dhi))
    cands.sort(key=lambda c: c[0] + c[1])
    best = None
    for k_lo, k_hi in cands:
        try:
            p = host_prep(x, edge_index, n_nodes, k_lo, k_hi, gw)
        except AssertionError:
            continue
        slots = p["W"] * p["K"]
        if best is None or slots < best["W"] * best["K"]:
            best = p
        if slots <= (E / NCORES) / 128 * 1.08:  # good enough
            break
    assert best is not None, "no feasible packing found"
    return best


# ---------------------------------------------------------------------------
# Bass program
# ---------------------------------------------------------------------------

def build_program(cfg):
    import os
    KSTOP = int(os.environ.get("KSTOP", "0"))  # debug: 1=phaseA 2=+AG1 3=+edge1 4=+phaseC+AG2
    import concourse.bacc as bacc
    import concourse.bass as bass
    import concourse.mybir as mybir
    from concourse import tile

    f32 = mybir.dt.float32
    f16 = mybir.dt.float16
    i16 = mybir.dt.int16
    AF = mybir.ActivationFunctionType
    OP = mybir.AluOpType

    W, P, K, k_lo, k_hi, gw = cfg["W"], cfg["P"], cfg["K"], cfg["k_lo"], cfg["k_hi"], cfg["gw"]
    hi_exists = cfg["hi_exists"]
    G = W // gw
    T = gw * K                  # tiles per group
    GCAP = 1024                 # max gather descriptors per SWDGE call
    NLO = gw * k_lo * 128       # lo gather idxs per group
    NHI = gw * k_hi * 128
    P_ALL = P * NCORES

    nc = bacc.Bacc("TRN2", target_bir_lowering=False, debug=False, num_devices=NCORES,
                   dynamic_dma_scratch_size=49152)

    # ---- external inputs ----
    xT = nc.dram_tensor("xT", [IN_CH, P], f16, kind="ExternalInput")
    w1e = nc.dram_tensor("w1e", [IN_CH, 136], f16, kind="ExternalInput")
    w2e = nc.dram_tensor("w2e", [HC1, 66], f32, kind="ExternalInput")
    idx_lo_d = nc.dram_tensor("idx_lo", [128, W * k_lo * 8], i16, kind="ExternalInput")
    if hi_exists:
        idx_hi_d = nc.dram_tensor("idx_hi", [128, W * k_hi * 8], i16, kind="ExternalInput")
    u8 = mybir.dt.uint8
    dc_d = nc.dram_tensor("dc", [128, W * K], f16, kind="ExternalInput")
    dr_d = nc.dram_tensor("dr", [W * K, 128], f16, kind="ExternalInput")
    iota_c_d = nc.dram_tensor("iota_c", [128, 1], f16, kind="ExternalInput")
    iota_r_d = nc.dram_tensor("iota_r", [128, 128], f16, kind="ExternalInput")
    ident_d = nc.dram_tensor("ident", [128, 128], f16, kind="ExternalInput")
    b1_d = nc.dram_tensor("b1", [128, HC1], f32, kind="ExternalInput")
    b2_d = nc.dram_tensor("b2", [128, C2], f32, kind="ExternalInput")
    out_d = nc.dram_tensor("out", [P, C2], f32, kind="ExternalOutput")

    with tile.TileContext(nc) as tc:
        with (
            tc.tile_pool(name="const", bufs=1) as cpool,
            tc.tile_pool(name="xc", bufs=3) as xcpool,
            tc.tile_pool(name="rows", bufs=3) as rowpool,
            tc.tile_pool(name="gather", bufs=2) as gpool,
            tc.tile_pool(name="onehot", bufs=2) as opool,
            tc.tile_pool(name="rmat", bufs=2) as rpool,
            tc.tile_pool(name="scal", bufs=3) as spool,
            tc.tile_pool(name="idx", bufs=3) as ipool,
            tc.tile_pool(name="epi", bufs=3) as epool,
            tc.tile_pool(name="psA", bufs=2, space="PSUM") as psA,
            tc.tile_pool(name="psW", bufs=4, space="PSUM") as psW,
            tc.tile_pool(name="psad", bufs=2, space="PSUM") as psad,
            tc.tile_pool(name="dram", bufs=1, space="DRAM") as dpool,
        ):
            # ---- constants to SBUF ----
            w1e_sb = cpool.tile([IN_CH, 136], f16, tag="w1e")
            nc.sync.dma_start(w1e_sb[:], w1e[:])
            w2e_sb = cpool.tile([HC1, 66], f32, tag="w2e")
            nc.sync.dma_start(w2e_sb[:], w2e[:])
            w2e_f16 = cpool.tile([HC1, 66], f16, tag="w2e16")
            nc.vector.tensor_copy(w2e_f16[:], w2e_sb[:])
            iota_c = cpool.tile([128, 1], f16, tag="iota_c")
            nc.sync.dma_start(iota_c[:], iota_c_d[:])
            iota_r = cpool.tile([128, 128], f16, tag="iota_r")
            nc.sync.dma_start(iota_r[:], iota_r_d[:])
            # iota_part[p, :] == p, packed f16 (keeps the sm build in 2x DVE mode)
            iota_part = cpool.tile([128, T * 128], f16, tag="iota_part")
            nc.scalar.activation(iota_part[:], iota_c[:].broadcast_to([128, T * 128]),
                                 AF.Copy)
            ident = cpool.tile([128, 128], f16, tag="ident")
            nc.sync.dma_start(ident[:], ident_d[:])
            b1_sb = cpool.tile([128, HC1], f32, tag="b1")
            nc.sync.dma_start(b1_sb[:], b1_d[:])
            b2_sb = cpool.tile([128, C2], f32, tag="b2")
            nc.sync.dma_start(b2_sb[:], b2_d[:])
            adsb1 = cpool.tile([128, W, H1], f16, tag="adsb1")
            adsb2 = cpool.tile([128, W, H2], f16, tag="adsb2")
            x2T = cpool.tile([128, P], f16, tag="x2T")

            # ---- DRAM tables ----
            tab1_slice = dpool.tile([P, ROW1], f16, tag="t1s")
            tab1_full = dpool.tile([P_ALL, ROW1], f16, tag="t1f", addr_space="Shared")
            tab2_slice = dpool.tile([P, ROW2], f16, tag="t2s")
            tab2_full = dpool.tile([P_ALL, ROW2], f16, tag="t2f", addr_space="Shared")

            # ================= phase A: layer-1 node matmul =================
            import os as _os2
            _kpha = _os2.environ.get("KPHA") == "1"  # debug: no matmuls in phase A
            for w in range(W):
                xc = xcpool.tile([IN_CH, 128], f16, tag="xc")
                nc.sync.dma_start(xc[:], xT[:, w * 128:(w + 1) * 128])
                rows = rowpool.tile([128, ROW1], f16, tag="rows1")
                if _kpha:
                    nc.vector.memset(rows[:], 0.25)
                    nc.vector.memset(adsb1[:, w, :], 0.25)
                else:
                    ps = psA.tile([128, 136], f32, tag="ps_node")
                    nc.tensor.matmul(ps[:], lhsT=xc[:], rhs=w1e_sb[:], start=True, stop=True)
                    nc.vector.memset(rows[:, 136:], 0.0)
                    nc.vector.tensor_copy(rows[:, 0:136], ps[:])
                    nc.vector.tensor_copy(adsb1[:, w, :], ps[:, 132:136])
                nc.sync.dma_start(tab1_slice[w * 128:(w + 1) * 128, :], rows[:])
            if KSTOP != 1:
                nc.gpsimd.collective_compute(
                    "AllGather", mybir.AluOpType.bypass,
                    replica_groups=[list(range(NCORES))],
                    ins=[tab1_slice.opt()], outs=[tab1_full.opt()],
                )

            # ================= edge phase (shared for both layers) ==========
            def edge_phase(layer):
                import os as _os
                _ked = int(_os.environ.get("KEDGE", "9"))
                _nog = _os.environ.get("KNOGATHER") == "1"

                if layer == 1:
                    table, row, heads, ch = tab1_full, ROW1, H1, C1
                    adsb = adsb1
                    a_s_off, a_d_off = 128, 132
                    rcols = HC1 + H1  # 132
                else:
                    table, row, heads, ch = tab2_full, ROW2, H2, C2
                    adsb = adsb2
                    a_s_off, a_d_off = 64, 65
                    rcols = C2 + H2  # 65
                hc = heads * ch
                nh = heads  # scalar cols per tile
                for g in range(G):
                    w0 = g * gw
                    # --- gather ---
                    Gt = gpool.tile([128, T, row], f16, tag=f"G{layer}")
                    il = ipool.tile([128, NLO // 16], i16, tag="il")
                    nc.sync.dma_start(il[:], idx_lo_d[:, g * (NLO // 16):(g + 1) * (NLO // 16)])
                    if not _nog:
                        for off in range(0, NLO, GCAP):
                            sz = min(GCAP, NLO - off)
                            nc.gpsimd.dma_gather(
                                out_ap=Gt[:, off // 128:(off + sz) // 128, :],
                                in_ap=table[0:min(HALF, P_ALL), :],
                                idxs_ap=il[:, off // 16:(off + sz) // 16],
                                num_idxs=sz, num_idxs_reg=sz,
                                elem_size=row)
                    else:
                        nc.vector.memset(Gt[:].rearrange("p t r -> p (t r)"), 0.25)
                    if hi_exists and k_hi > 0:
                        ih = ipool.tile([128, NHI // 16], i16, tag="ih")
                        nc.sync.dma_start(ih[:], idx_hi_d[:, g * (NHI // 16):(g + 1) * (NHI // 16)])
                        if not _nog:
                            for off in range(0, NHI, GCAP):
                                sz = min(GCAP, NHI - off)
                                nc.gpsimd.dma_gather(
                                    out_ap=Gt[:, gw * k_lo + off // 128:gw * k_lo + (off + sz) // 128, :],
                                    in_ap=table[HALF:P_ALL, :],
                                    idxs_ap=ih[:, off // 16:(off + sz) // 16],
                                    num_idxs=sz, num_idxs_reg=sz,
                                    elem_size=row)
                    # --- one-hot builds ---
                    if _ked < 2:
                        dmy = spool.tile([128, T, row], f16, tag="dmyG")
                        nc.vector.tensor_copy(dmy[:], Gt[:])
                        continue
                    dc_sb = ipool.tile([128, T], f16, tag="dc")
                    nc.sync.dma_start(dc_sb[:], dc_d[:, g * T:(g + 1) * T])
                    dr_rep = ipool.tile([128, T * 128], f16, tag="dr_rep")
                    nc.sync.dma_start(
                        dr_rep[:],
                        dr_d[g * T:(g + 1) * T, :]
                        .rearrange("(one a) b -> one (a b)", one=1)
                        .partition_broadcast(128).opt())
                    # expand dc on the (idle) Act engine so the DVE compare
                    # sees packed 2-byte operands and runs in 2x mode
                    dc_exp = opool.tile([128, T, 128], f16, tag="dc_exp")
                    nc.scalar.activation(
                        dc_exp[:],
                        dc_sb[:].rearrange("p (t one) -> p t one", one=1).broadcast_to([128, T, 128]),
                        AF.Copy)
                    e1 = opool.tile([128, T, 128], f16, tag="e1")
                    nc.vector.tensor_tensor(
                        e1[:],
                        dc_exp[:],
                        iota_r[:].rearrange("p (one x) -> p one x", one=1).broadcast_to([128, T, 128]),
                        OP.is_equal)
                    sm = opool.tile([128, T, 128], f16, tag="sm")
                    nc.vector.tensor_tensor(
                        sm[:].rearrange("p t j -> p (t j)"),
                        iota_part[:],
                        dr_rep[:],
                        OP.is_equal)
                    if _ked < 3:
                        continue
                    # --- a_d broadcast matmuls ---
                    pad = psad.tile([128, T * nh], f32, tag="pad")
                    for t in range(T):
                        w = w0 + (t // k_lo if t < gw * k_lo else (t - gw * k_lo) // k_hi)
                        nc.tensor.matmul(pad[:, t * nh:(t + 1) * nh],
                                         lhsT=sm[:, t, :], rhs=adsb[:, w, :],
                                         start=True, stop=True)
                    if _ked < 4:
                        continue
                    # --- per-edge scalars ---
                    z = spool.tile([128, T, nh], f32, tag="z")
                    nc.vector.tensor_add(z[:], Gt[:, :, a_s_off:a_s_off + nh],
                                         pad[:].rearrange("p (t h) -> p t h", h=nh))
                    z2 = spool.tile([128, T, nh], f32, tag="z2")
                    nc.vector.tensor_scalar_mul(z2[:], z[:], NEG_SLOPE)
                    nc.vector.tensor_max(z[:], z[:], z2[:])
                    psc = spool.tile([128, T, nh], f16, tag="psc")
                    nc.scalar.activation(psc[:], z[:], AF.Exp)
                    if _ked < 5:
                        continue
                    # --- R = [p*h | p] ---
                    # expand psc per-head on the Act engine so the DVE mul is
                    # a single packed-f16 2x-mode op
                    R = rpool.tile([128, T, rcols], f16, tag=f"R")
                    for h in range(heads):
                        nc.vector.tensor_mul(
                            R[:, :, h * ch:(h + 1) * ch],
                            Gt[:, :, h * ch:(h + 1) * ch],
                            psc[:, :, h:h + 1].broadcast_to([128, T, ch]))
                    nc.vector.tensor_copy(R[:, :, hc:hc + nh], psc[:])
                    nc.vector.tensor_copy(R[:, :, hc:hc + nh], psc[:])
                    if _ked < 6:
                        continue
                    # --- segment-sum matmuls ---
                    pw = [psW.tile([128, rcols], f32, tag="psW", name=f"pw{g}_{wi}")
                          for wi in range(gw)]
                    for t in range(T):
                        if t < gw * k_lo:
                            wi, first = divmod(t, k_lo)
                            is_first = first == 0
                            is_last = (first == k_lo - 1) and k_hi == 0
                        else:
                            wi, r = divmod(t - gw * k_lo, k_hi)
                            is_first = False
                            is_last = r == k_hi - 1
                        nc.tensor.matmul(pw[wi][:], lhsT=e1[:, t, :], rhs=R[:, t, :],
                                         start=is_first, stop=is_last)
                    # --- epilogue per window ---
                    import os as _os
                    _epi = int(_os.environ.get("KEPI", "0"))  # 6=no epilogue, 7=no transpose
                    for wi in range(gw):
                        if _epi == 6:
                            dummy = epool.tile([128, rcols], f16, tag="dummy")
                            nc.vector.tensor_copy(dummy[:], pw[wi][:])
                            continue
                        w = w0 + wi
                        den = epool.tile([128, nh], f32, tag="den")
                        nc.vector.tensor_scalar_add(den[:], pw[wi][:, hc:hc + nh], EPS)
                        rec = epool.tile([128, nh], f32, tag="rec")
                        nc.vector.reciprocal(rec[:], den[:])
                        o = epool.tile([128, hc], f32, tag="o")
                        nc.vector.tensor_mul(
                            o[:].rearrange("p (h c) -> p h c", h=heads),
                            pw[wi][:, 0:hc].rearrange("p (h c) -> p h c", h=heads),
                            rec[:].broadcast_to([128, heads, ch]))
                        if layer == 1:
                            nc.vector.tensor_add(o[:], o[:], b1_sb[:])
                            neg = epool.tile([128, hc], f32, tag="neg")
                            nc.vector.tensor_scalar_min(neg[:], o[:], 0.0)
                            nc.scalar.activation(neg[:], neg[:], AF.Exp)
                            pos = epool.tile([128, hc], f32, tag="pos")
                            nc.vector.tensor_scalar_max(pos[:], o[:], 0.0)
                            act = epool.tile([128, hc], f16, tag="act")
                            # act = (exp(neg) - 1) + pos  (ELU), fused on DVE
                            nc.vector.scalar_tensor_tensor(
                                act[:], neg[:], -1.0, pos[:],
                                OP.add, OP.add)
                            if _epi == 7:
                                nc.vector.tensor_copy(x2T[:, w * 128:(w + 1) * 128], act[:])
                            else:
                                psT = psA.tile([128, 128], f16, tag="ps_node")
                                nc.tensor.transpose(psT[:], act[:], ident[:])
                                nc.vector.tensor_copy(x2T[:, w * 128:(w + 1) * 128], psT[:])
                        else:
                            nc.vector.tensor_add(o[:], o[:], b2_sb[:])
                            nc.sync.dma_start(out_d[w * 128:(w + 1) * 128, :], o[:])

            if KSTOP in (0, 3, 4):
                edge_phase(1)

            # ================= phase C: layer-2 node matmul =================
            for w in range(W if KSTOP in (0, 4) else 0):
                ps = psA.tile([128, 66], f32, tag="ps_node")
                nc.tensor.matmul(ps[:], lhsT=x2T[:, w * 128:(w + 1) * 128],
                                 rhs=w2e_f16[:], start=True, stop=True)
                rows = rowpool.tile([128, ROW2], f16, tag="rows2")
                nc.vector.memset(rows[:, 66:], 0.0)
                nc.vector.tensor_copy(rows[:, 0:66], ps[:])
                nc.vector.tensor_copy(adsb2[:, w, :], ps[:, 65:66])
                nc.sync.dma_start(tab2_slice[w * 128:(w + 1) * 128, :], rows[:])
            if KSTOP in (0, 4):
                nc.gpsimd.collective_compute(
                    "AllGather", mybir.AluOpType.bypass,
                    replica_groups=[list(range(NCORES))],
                    ins=[tab2_slice.opt()], outs=[tab2_full.opt()],
                )

            if KSTOP == 0:
                edge_phase(2)

    nc.compile()
    return nc


# ---------------------------------------------------------------------------
# Entry point
# ---------------------------------------------------------------------------

_CACHE = {}


def _prepare(inputs):
    x = np.ascontiguousarray(np.asarray(inputs["x"], np.float32))
    ei = np.asarray(inputs["edge_index"])
    n_nodes = x.shape[0]
    return pick_config(x, ei, n_nodes)


def _weights_ext(inputs):
    W1 = np.asarray(inputs["W1"], np.float32)
    as1 = np.asarray(inputs["att_src1"], np.float32)
    ad1 = np.asarray(inputs["att_dst1"], np.float32)
    W2 = np.asarray(inputs["W2"], np.float32)
    as2 = np.asarray(inputs["att_src2"], np.float32)
    ad2 = np.asarray(inputs["att_dst2"], np.float32)
    As = np.zeros((HC1, H1), np.float32)
    Ad = np.zeros((HC1, H1), np.float32)
    for h in range(H1):
        As[h * C1:(h + 1) * C1, h] = as1[0, h]
        Ad[h * C1:(h + 1) * C1, h] = ad1[0, h]
    w1e = np.concatenate([W1, W1 @ As, W1 @ Ad], axis=1)           # [128,136]
    w2e = np.concatenate([W2, W2 @ as2[0].T, W2 @ ad2[0].T], axis=1)  # [128,66]
    return np.ascontiguousarray(w1e).astype(np.float16), np.ascontiguousarray(w2e)


def kernel(**inputs):
    from concourse.bass_utils import run_bass_kernel_spmd

    prep = _prepare(inputs)
    key = (prep["W"], prep["K"], prep["k_lo"], prep["k_hi"], prep["gw"], prep["hi_exists"])
    if key not in _CACHE:
        _CACHE[key] = build_program(dict(
            W=prep["W"], P=prep["P"], K=prep["K"], k_lo=prep["k_lo"],
            k_hi=prep["k_hi"], gw=prep["gw"], hi_exists=prep["hi_exists"]))
    nc = _CACHE[key]

    in_maps = build_in_maps(inputs, prep)
    res = run_bass_kernel_spmd(nc, in_maps, core_ids=list(range(NCORES)))
    return assemble_output(res.results, prep)


def build_in_maps(inputs, prep):
    x = np.ascontiguousarray(np.asarray(inputs["x"], np.float32))
    b1 = np.tile(np.asarray(inputs["b1"], np.float32).reshape(1, HC1), (128, 1))
    b2 = np.tile(np.asarray(inputs["b2"], np.float32).reshape(1, C2), (128, 1))
    w1e, w2e = _weights_ext(inputs)
    n_nodes, P, W = prep["n_nodes"], prep["P"], prep["W"]
    pi = prep["pi"]
    iota_c = np.arange(128, dtype=np.float16).reshape(128, 1)
    iota_r = np.tile(np.arange(128, dtype=np.float16), (128, 1))
    ident = np.eye(128, dtype=np.float16)
    in_maps = []
    # xT_pad per core: columns = padded slots
    xT_all = np.zeros((NCORES, IN_CH, P), np.float16)
    node_ids = np.arange(n_nodes)
    c_of = pi // P
    col = pi % P
    xT_all[c_of, :, col] = x[node_ids]  # fancy: sets [ch] vectors
    for c in range(NCORES):
        m = prep["meta"][c]
        im = dict(
            xT=np.ascontiguousarray(xT_all[c]),
            w1e=w1e, w2e=w2e,
            idx_lo=np.ascontiguousarray(m["idx_lo"]),
            dc=np.ascontiguousarray(m["dc"]),
            dr=np.ascontiguousarray(m["dr"]),
            iota_c=iota_c, iota_r=iota_r, ident=ident,
            b1=b1, b2=b2,
        )
        if prep["hi_exists"]:
            im["idx_hi"] = np.ascontiguousarray(m["idx_hi"])
        in_maps.append(im)
    return in_maps


def assemble_output(results, prep):
    P, n_nodes = prep["P"], prep["n_nodes"]
    full = np.concatenate([results[c]["out"] for c in range(NCORES)], axis=0)
    return np.ascontiguousarray(full[prep["pi"]]).astype(np.float32)



# revision 52
# speedup vs baseline: 17.2168x; 16.6957x over previous
"""Trainium2 Bass kernel for a 2-layer GAT encoder (edge-softmax message passing).

Strategy (8 NeuronCores, SPMD single program):
- dst-node partition across cores; host packs each core's dst nodes into
  fixed-count "windows" (<=128 nodes each) and edges into fixed-count
  128-edge tiles per window (K_LO tiles for src in the low half of the
  padded node space, K_HI for the high half -- dma_gather indices are int16).
- Node phase: h_ext = x @ [W | W@att_src | W@att_dst] per core slice
  (node-major matmuls with x^T chunks stationary), fp16 row table written to
  DRAM, AllGather -> full table on every core.
- Edge phase per 128-edge tile: dma_gather rows by src; the dst one-hot e1
  is built on DVE by an iota-vs-dstrel compare (packed fp16 operands for 2x
  DVE mode, dst-slot column pre-broadcast on the idle Act engine); its
  transpose sm is shipped precomputed from the host (same DMA bytes as
  shipping the slot ids, zero DVE build cost); per-edge a_d via sm matmul;
  p = exp(leaky_relu(a_s+a_d)) (softmax shift invariance makes the
  segment-max subtraction unnecessary); segment-sum of [p*h | p] via one-hot
  matmul accumulated in PSUM per window; epilogue divides and applies
  bias/ELU (ELU tail fused via scalar_tensor_tensor).
- Node-phase matmuls run in fp16 (x and W1 shipped as fp16); x chunks are
  loaded 4 windows per DMA and table rows written 2 windows per DMA to
  relieve the HWDGE dispatch queue; L2 outputs written 2 windows per DMA.
- sm/idx tiles use deep pools so their loads prefetch during the AllGather;
  3-deep gather pool pipelines groups across the edge phase.
- Output rows are window-padded; host de-permutes to the original node order.
"""

import numpy as np

NCORES = 8
HALF = 32768          # int16 gather index limit
ROW1 = 256            # fp16 elems per layer-1 table row (512B): h128|a_s4|a_d4|pad
ROW2 = 128            # fp16 elems per layer-2 table row (256B): h64|a_s|a_d|pad
H1, C1 = 4, 32
H2, C2 = 1, 64
IN_CH = 128
HC1 = H1 * C1         # 128
NEG_SLOPE = 0.2
EPS = 1e-16


# ---------------------------------------------------------------------------
# Host-side preprocessing
# ---------------------------------------------------------------------------

def _pack_windows(src, dst, n_nodes, k_lo, k_hi, boundary):
    """Greedy-pack each core's dst nodes into windows (<=128 nodes, <=k_lo
    lo-tiles, <=k_hi hi-tiles). Returns per-core list of windows; each window
    is (node_lo, node_hi, lo_edge_srcs, hi_edge_srcs, lo_dstrel, hi_dstrel).
    Edges must be sorted by dst."""
    per_core = n_nodes // NCORES
    cores = []
    # bucket edges by dst once
    order = np.argsort(dst, kind="stable")
    src_s, dst_s = src[order], dst[order]
    # node -> edge range (dst-sorted)
    counts = np.bincount(dst_s, minlength=n_nodes)
    starts = np.concatenate([[0], np.cumsum(counts)])
    for c in range(NCORES):
        lo_n, hi_n = c * per_core, (c + 1) * per_core
        wins = []
        n = lo_n
        while n < hi_n:
            w_nodes = 0
            w_lo = []
            w_hi = []
            w_lo_dr = []
            w_hi_dr = []
            base = n
            while n < hi_n and w_nodes < 128:
                e0, e1 = starts[n], starts[n + 1]
                es = src_s[e0:e1]
                lo_m = es < boundary
                nlo = int(lo_m.sum())
                nhi = es.shape[0] - nlo
                cur_lo = sum(len(a) for a in w_lo)
                cur_hi = sum(len(a) for a in w_hi)
                if cur_lo + nlo > k_lo * 128 or cur_hi + nhi > k_hi * 128:
                    break
                w_lo.append(es[lo_m])
                w_hi.append(es[~lo_m])
                w_lo_dr.append(np.full(nlo, w_nodes, np.int32))
                w_hi_dr.append(np.full(nhi, w_nodes, np.int32))
                w_nodes += 1
                n += 1
            assert w_nodes > 0, "single node exceeds tile budget"
            wins.append((base, n,
                         np.concatenate(w_lo) if w_lo else np.zeros(0, src.dtype),
                         np.concatenate(w_hi) if w_hi else np.zeros(0, src.dtype),
                         np.concatenate(w_lo_dr) if w_lo_dr else np.zeros(0, np.int32),
                         np.concatenate(w_hi_dr) if w_hi_dr else np.zeros(0, np.int32)))
        cores.append(wins)
    return cores


def host_prep(x, edge_index, n_nodes, k_lo, k_hi, gw):
    """Build the permutation, per-core metadata and index arrays."""
    src = np.ascontiguousarray(edge_index[0]).astype(np.int64)
    dst = np.ascontiguousarray(edge_index[1]).astype(np.int64)
    per_core = n_nodes // NCORES
    assert per_core * NCORES == n_nodes

    # fixpoint on the lo/hi boundary in *real node id* space (pi is monotone)
    boundary = min(n_nodes, HALF)
    for _ in range(6):
        cores = _pack_windows(src, dst, n_nodes, k_lo, k_hi, boundary)
        W = max(len(w) for w in cores)
        W = ((W + gw - 1) // gw) * gw  # pad to group multiple
        P = W * 128
        # pi: node -> padded slot id
        pi = np.zeros(n_nodes, np.int64)
        for c, wins in enumerate(cores):
            for w, (a, b, *_r) in enumerate(wins):
                ids = np.arange(a, b)
                pi[ids] = c * P + w * 128 + (ids - a)
        nb = int(np.searchsorted(pi, HALF))  # first node with pi >= HALF
        if nb == boundary or P * NCORES <= HALF:
            boundary = nb if P * NCORES > HALF else n_nodes
            break
        boundary = nb
    cores = _pack_windows(src, dst, n_nodes, k_lo, k_hi, boundary)
    W = max(len(w) for w in cores)
    W = ((W + gw - 1) // gw) * gw
    P = W * 128
    pi = np.zeros(n_nodes, np.int64)
    for c, wins in enumerate(cores):
        for w, (a, b, *_r) in enumerate(wins):
            ids = np.arange(a, b)
            pi[ids] = c * P + w * 128 + (ids - a)
    assert P * NCORES <= 65536, f"padded node space {P*NCORES} exceeds uint16 gather range"
    hi_exists = P * NCORES > HALF
    if not hi_exists:
        assert all(len(w[3]) == 0 for ws in cores for w in ws)

    K = k_lo + k_hi
    meta = []
    for c, wins in enumerate(cores):
        idx_lo = np.zeros((W, k_lo * 128), np.int16)
        idx_hi = np.zeros((W, k_hi * 128), np.int16)
        drel = np.full((W, K, 128), 255, np.int32)  # [window, tile-in-window, slot]
        for w, (a, b, lo_s, hi_s, lo_dr, hi_dr) in enumerate(wins):
            pl = pi[lo_s]
            assert (pl < HALF).all()
            idx_lo[w, :len(pl)] = pl.astype(np.int16)
            # lo tiles occupy tile slots [0, k_lo)
            dr_pad = np.full(k_lo * 128, 255, np.int32)
            dr_pad[:len(lo_dr)] = lo_dr
            drel[w, :k_lo] = dr_pad.reshape(k_lo, 128)
            if hi_exists:
                ph = pi[hi_s] - HALF
                assert (ph >= 0).all() and (ph < 32768).all()
                idx_hi[w, :len(ph)] = ph.astype(np.int16)
            dr_pad = np.full(k_hi * 128, 255, np.int32)
            dr_pad[:len(hi_dr)] = hi_dr
            drel[w, k_lo:] = dr_pad.reshape(k_hi, 128) if k_hi else drel[w, k_lo:]
        # group-tile order: per group: [lo tiles of gw windows][hi tiles of gw windows]
        G = W // gw
        tile_order = []  # (window, tile-in-window-index)
        for g in range(G):
            for w in range(g * gw, (g + 1) * gw):
                tile_order += [(w, t) for t in range(k_lo)]
            for w in range(g * gw, (g + 1) * gw):
                tile_order += [(w, k_lo + t) for t in range(k_hi)]
        to = np.array(tile_order)
        drel_t = drel[to[:, 0], to[:, 1]]            # [W*K, 128]
        # idx arrays in 16-partition wrapped layout: idx j -> [j%16, j//16]
        def wrap16(a):
            a = a.reshape(-1)
            # idx j lives at [j%16, j//16], replicated into all 8 Q7 core
            # partition groups (HW reads each group independently)
            return np.ascontiguousarray(np.tile(a.reshape(-1, 16).T, (8, 1)))
        meta.append(dict(
            idx_lo=wrap16(idx_lo),
            idx_hi=wrap16(idx_hi) if hi_exists else None,
            dc=np.ascontiguousarray(drel_t.T).astype(np.float16),   # [128, W*K]
            # host-computed sm one-hot: sm[p, t*128+j] = (p == drel_t[t, j]);
            # same DMA bytes as shipping the slot ids broadcast, zero DVE build
            sm=np.ascontiguousarray(
                (np.arange(128, dtype=np.int32)[:, None, None] == drel_t[None, :, :])
                .astype(np.float16).reshape(128, -1)),               # [128, W*K*128]
        ))
    return dict(cores=cores, pi=pi, W=W, P=P, K=K, k_lo=k_lo, k_hi=k_hi, gw=gw,
                hi_exists=hi_exists, meta=meta, n_nodes=n_nodes, per_core=per_core)


def pick_config(x, edge_index, n_nodes):
    """Try candidate (k_lo, k_hi) packings, return the prep with fewest tiles."""
    gw = 2
    E = edge_index.shape[1]
    lam = E / n_nodes * 128
    base_lo = max(int(np.ceil(lam * 0.64 / 128)), 1)
    base_hi = max(int(np.ceil(lam * 0.36 / 128)), 0)
    cands = []
    for dlo in (-1, 0, 1, 2):
        for dhi in (-1, 0, 1, 2):
            if base_lo + dlo >= 1 and base_hi + dhi >= 0:
                cands.append((base_lo + dlo, base_hi + dhi))
    cands.sort(key=lambda c: c[0] + c[1])
    best = None
    for k_lo, k_hi in cands:
        try:
            p = host_prep(x, edge_index, n_nodes, k_lo, k_hi, gw)
        except AssertionError:
            continue
        slots = p["W"] * p["K"]
        if best is None or slots < best["W"] * best["K"]:
            best = p
        if slots <= (E / NCORES) / 128 * 1.08:  # good enough
            break
    assert best is not None, "no feasible packing found"
    return best


# ---------------------------------------------------------------------------
# Bass program
# ---------------------------------------------------------------------------

def build_program(cfg):
    import os
    KSTOP = int(os.environ.get("KSTOP", "0"))  # debug: 1=phaseA 2=+AG1 3=+edge1 4=+phaseC+AG2
    import concourse.bacc as bacc
    import concourse.bass as bass
    import concourse.mybir as mybir
    from concourse import tile

    f32 = mybir.dt.float32
    f16 = mybir.dt.float16
    i16 = mybir.dt.int16
    AF = mybir.ActivationFunctionType
    OP = mybir.AluOpType

    W, P, K, k_lo, k_hi, gw = cfg["W"], cfg["P"], cfg["K"], cfg["k_lo"], cfg["k_hi"], cfg["gw"]
    hi_exists = cfg["hi_exists"]
    G = W // gw
    T = gw * K                  # tiles per group
    GCAP = 1024                 # max gather descriptors per SWDGE call
    NLO = gw * k_lo * 128       # lo gather idxs per group
    NHI = gw * k_hi * 128
    P_ALL = P * NCORES

    nc = bacc.Bacc("TRN2", target_bir_lowering=False, debug=False, num_devices=NCORES,
                   dynamic_dma_scratch_size=40960)

    # ---- external inputs ----
    xT = nc.dram_tensor("xT", [IN_CH, P], f16, kind="ExternalInput")
    w1e = nc.dram_tensor("w1e", [IN_CH, 136], f16, kind="ExternalInput")
    w2e = nc.dram_tensor("w2e", [HC1, 66], f32, kind="ExternalInput")
    idx_lo_d = nc.dram_tensor("idx_lo", [128, W * k_lo * 8], i16, kind="ExternalInput")
    if hi_exists:
        idx_hi_d = nc.dram_tensor("idx_hi", [128, W * k_hi * 8], i16, kind="ExternalInput")
    u8 = mybir.dt.uint8
    dc_d = nc.dram_tensor("dc", [128, W * K], f16, kind="ExternalInput")
    smh_d = nc.dram_tensor("smh", [128, W * K * 128], f16, kind="ExternalInput")
    iota_r_d = nc.dram_tensor("iota_r", [128, 128], f16, kind="ExternalInput")
    ident_d = nc.dram_tensor("ident", [128, 128], f16, kind="ExternalInput")
    b1_d = nc.dram_tensor("b1", [128, HC1], f32, kind="ExternalInput")
    b2_d = nc.dram_tensor("b2", [128, C2], f32, kind="ExternalInput")
    out_d = nc.dram_tensor("out", [P, C2], f32, kind="ExternalOutput")

    with tile.TileContext(nc) as tc:
        with (
            tc.tile_pool(name="const", bufs=1) as cpool,
            tc.tile_pool(name="xc", bufs=3) as xcpool,
            tc.tile_pool(name="rows", bufs=3) as rowpool,
            tc.tile_pool(name="gather", bufs=3) as gpool,
            tc.tile_pool(name="onehot", bufs=2) as opool,
            tc.tile_pool(name="rmat", bufs=2) as rpool,
            tc.tile_pool(name="scal", bufs=3) as spool,
            tc.tile_pool(name="idx", bufs=6) as ipool,
            tc.tile_pool(name="smp", bufs=3) as smpool,
            tc.tile_pool(name="epi", bufs=3) as epool,
            tc.tile_pool(name="psA", bufs=2, space="PSUM") as psA,
            tc.tile_pool(name="psW", bufs=4, space="PSUM") as psW,
            tc.tile_pool(name="psad", bufs=2, space="PSUM") as psad,
            tc.tile_pool(name="dram", bufs=1, space="DRAM") as dpool,
        ):
            # ---- constants to SBUF ----
            w1e_sb = cpool.tile([IN_CH, 136], f16, tag="w1e")
            nc.sync.dma_start(w1e_sb[:], w1e[:])
            w2e_sb = cpool.tile([HC1, 66], f32, tag="w2e")
            nc.sync.dma_start(w2e_sb[:], w2e[:])
            w2e_f16 = cpool.tile([HC1, 66], f16, tag="w2e16")
            nc.vector.tensor_copy(w2e_f16[:], w2e_sb[:])
            iota_r = cpool.tile([128, 128], f16, tag="iota_r")
            nc.sync.dma_start(iota_r[:], iota_r_d[:])
            ident = cpool.tile([128, 128], f16, tag="ident")
            nc.sync.dma_start(ident[:], ident_d[:])
            b1_sb = cpool.tile([128, HC1], f32, tag="b1")
            nc.sync.dma_start(b1_sb[:], b1_d[:])
            b2_sb = cpool.tile([128, C2], f32, tag="b2")
            nc.sync.dma_start(b2_sb[:], b2_d[:])
            adsb1 = cpool.tile([128, W, H1], f16, tag="adsb1")
            adsb2 = cpool.tile([128, W, H2], f16, tag="adsb2")
            x2T = cpool.tile([128, P], f16, tag="x2T")

            # ---- DRAM tables ----
            tab1_slice = dpool.tile([P, ROW1], f16, tag="t1s")
            tab1_full = dpool.tile([P_ALL, ROW1], f16, tag="t1f", addr_space="Shared")
            tab2_slice = dpool.tile([P, ROW2], f16, tag="t2s")
            tab2_full = dpool.tile([P_ALL, ROW2], f16, tag="t2f", addr_space="Shared")

            # ================= phase A: layer-1 node matmul =================
            import os as _os2
            _kpha = _os2.environ.get("KPHA") == "1"  # debug: no matmuls in phase A
            XCHUNK = 4
            for w in range(W):
                if w % XCHUNK == 0:
                    nw = min(XCHUNK, W - w)
                    xc = xcpool.tile([IN_CH, XCHUNK * 128], f16, tag="xc")
                    nc.sync.dma_start(xc[:, 0:nw * 128],
                                      xT[:, w * 128:(w + nw) * 128])
                xcv = xc[:, (w % XCHUNK) * 128:(w % XCHUNK + 1) * 128]
                if w % 2 == 0:
                    rows = rowpool.tile([128, 2, ROW1], f16, tag="rows1")
                rv = rows[:, w % 2, :]
                if _kpha:
                    nc.vector.memset(rv, 0.25)
                    nc.vector.memset(adsb1[:, w, :], 0.25)
                else:
                    ps = psA.tile([128, 136], f32, tag="ps_node")
                    nc.tensor.matmul(ps[:], lhsT=xcv, rhs=w1e_sb[:], start=True, stop=True)
                    nc.vector.memset(rv[:, 136:], 0.0)
                    nc.vector.tensor_copy(rv[:, 0:136], ps[:])
                    nc.vector.tensor_copy(adsb1[:, w, :], ps[:, 132:136])
                if w % 2 == 1:
                    nc.sync.dma_start(
                        tab1_slice[(w - 1) * 128:(w + 1) * 128, :]
                        .rearrange("(i p) r -> p i r", p=128),
                        rows[:])
            if KSTOP != 1:
                nc.gpsimd.collective_compute(
                    "AllGather", mybir.AluOpType.bypass,
                    replica_groups=[list(range(NCORES))],
                    ins=[tab1_slice.opt()], outs=[tab1_full.opt()],
                )

            # ================= edge phase (shared for both layers) ==========
            def edge_phase(layer):
                import os as _os
                _ked = int(_os.environ.get("KEDGE", "9"))
                _nog = _os.environ.get("KNOGATHER") == "1"

                if layer == 1:
                    table, row, heads, ch = tab1_full, ROW1, H1, C1
                    adsb = adsb1
                    a_s_off, a_d_off = 128, 132
                    rcols = HC1 + H1  # 132
                else:
                    table, row, heads, ch = tab2_full, ROW2, H2, C2
                    adsb = adsb2
                    a_s_off, a_d_off = 64, 65
                    rcols = C2 + H2  # 65
                hc = heads * ch
                nh = heads  # scalar cols per tile
                for g in range(G):
                    w0 = g * gw
                    # --- gather ---
                    Gt = gpool.tile([128, T, row], f16, tag=f"G{layer}")
                    il = ipool.tile([128, NLO // 16], i16, tag="il")
                    nc.sync.dma_start(il[:], idx_lo_d[:, g * (NLO // 16):(g + 1) * (NLO // 16)])
                    if not _nog:
                        for off in range(0, NLO, GCAP):
                            sz = min(GCAP, NLO - off)
                            nc.gpsimd.dma_gather(
                                out_ap=Gt[:, off // 128:(off + sz) // 128, :],
                                in_ap=table[0:min(HALF, P_ALL), :],
                                idxs_ap=il[:, off // 16:(off + sz) // 16],
                                num_idxs=sz, num_idxs_reg=sz,
                                elem_size=row)
                    else:
                        nc.vector.memset(Gt[:].rearrange("p t r -> p (t r)"), 0.25)
                    if hi_exists and k_hi > 0:
                        ih = ipool.tile([128, NHI // 16], i16, tag="ih")
                        nc.sync.dma_start(ih[:], idx_hi_d[:, g * (NHI // 16):(g + 1) * (NHI // 16)])
                        if not _nog:
                            for off in range(0, NHI, GCAP):
                                sz = min(GCAP, NHI - off)
                                nc.gpsimd.dma_gather(
                                    out_ap=Gt[:, gw * k_lo + off // 128:gw * k_lo + (off + sz) // 128, :],
                                    in_ap=table[HALF:P_ALL, :],
                                    idxs_ap=ih[:, off // 16:(off + sz) // 16],
                                    num_idxs=sz, num_idxs_reg=sz,
                                    elem_size=row)
                    # --- one-hot builds ---
                    if _ked < 2:
                        dmy = spool.tile([128, T, row], f16, tag="dmyG")
                        nc.vector.tensor_copy(dmy[:], Gt[:])
                        continue
                    dc_sb = ipool.tile([128, T], f16, tag="dc")
                    nc.sync.dma_start(dc_sb[:], dc_d[:, g * T:(g + 1) * T])
                    sm = smpool.tile([128, T, 128], f16, tag="sm")
                    nc.sync.dma_start(
                        sm[:].rearrange("p t j -> p (t j)"),
                        smh_d[:, g * T * 128:(g + 1) * T * 128])
                    # expand dc on the (idle) Act engine so the DVE compare
                    # sees packed 2-byte operands and runs in 2x mode
                    dc_exp = opool.tile([128, T, 128], f16, tag="dc_exp")
                    nc.scalar.activation(
                        dc_exp[:],
                        dc_sb[:].rearrange("p (t one) -> p t one", one=1).broadcast_to([128, T, 128]),
                        AF.Copy)
                    e1 = opool.tile([128, T, 128], f16, tag="e1")
                    nc.vector.tensor_tensor(
                        e1[:],
                        dc_exp[:],
                        iota_r[:].rearrange("p (one x) -> p one x", one=1).broadcast_to([128, T, 128]),
                        OP.is_equal)
                    if _ked < 3:
                        continue
                    # --- a_d broadcast matmuls ---
                    pad = psad.tile([128, T * nh], f32, tag="pad")
                    for t in range(T):
                        w = w0 + (t // k_lo if t < gw * k_lo else (t - gw * k_lo) // k_hi)
                        nc.tensor.matmul(pad[:, t * nh:(t + 1) * nh],
                                         lhsT=sm[:, t, :], rhs=adsb[:, w, :],
                                         start=True, stop=True)
                    if _ked < 4:
                        continue
                    # --- per-edge scalars ---
                    z = spool.tile([128, T, nh], f32, tag="z")
                    nc.vector.tensor_add(z[:], Gt[:, :, a_s_off:a_s_off + nh],
                                         pad[:].rearrange("p (t h) -> p t h", h=nh))
                    z2 = spool.tile([128, T, nh], f32, tag="z2")
                    nc.vector.tensor_scalar_mul(z2[:], z[:], NEG_SLOPE)
                    nc.vector.tensor_max(z[:], z[:], z2[:])
                    psc = spool.tile([128, T, nh], f16, tag="psc")
                    nc.scalar.activation(psc[:], z[:], AF.Exp)
                    if _ked < 5:
                        continue
                    # --- R = [p*h | p] ---
                    # expand psc per-head on the Act engine so the DVE mul is
                    # a single packed-f16 2x-mode op
                    R = rpool.tile([128, T, rcols], f16, tag=f"R")
                    for h in range(heads):
                        nc.vector.tensor_mul(
                            R[:, :, h * ch:(h + 1) * ch],
                            Gt[:, :, h * ch:(h + 1) * ch],
                            psc[:, :, h:h + 1].broadcast_to([128, T, ch]))
                    nc.vector.tensor_copy(R[:, :, hc:hc + nh], psc[:])
                    nc.vector.tensor_copy(R[:, :, hc:hc + nh], psc[:])
                    if _ked < 6:
                        continue
                    # --- segment-sum matmuls ---
                    pw = [psW.tile([128, rcols], f32, tag="psW", name=f"pw{g}_{wi}")
                          for wi in range(gw)]
                    for t in range(T):
                        if t < gw * k_lo:
                            wi, first = divmod(t, k_lo)
                            is_first = first == 0
                            is_last = (first == k_lo - 1) and k_hi == 0
                        else:
                            wi, r = divmod(t - gw * k_lo, k_hi)
                            is_first = False
                            is_last = r == k_hi - 1
                        nc.tensor.matmul(pw[wi][:], lhsT=e1[:, t, :], rhs=R[:, t, :],
                                         start=is_first, stop=is_last)
                    # --- epilogue per window ---
                    import os as _os
                    _epi = int(_os.environ.get("KEPI", "0"))  # 6=no epilogue, 7=no transpose
                    for wi in range(gw):
                        if _epi == 6:
                            dummy = epool.tile([128, rcols], f16, tag="dummy")
                            nc.vector.tensor_copy(dummy[:], pw[wi][:])
                            continue
                        w = w0 + wi
                        den = epool.tile([128, nh], f32, tag="den")
                        nc.vector.tensor_scalar_add(den[:], pw[wi][:, hc:hc + nh], EPS)
                        rec = epool.tile([128, nh], f32, tag="rec")
                        nc.vector.reciprocal(rec[:], den[:])
                        if layer == 1:
                            o = epool.tile([128, hc], f32, tag="o")
                            nc.vector.tensor_mul(
                                o[:].rearrange("p (h c) -> p h c", h=heads),
                                pw[wi][:, 0:hc].rearrange("p (h c) -> p h c", h=heads),
                                rec[:].broadcast_to([128, heads, ch]))
                            nc.vector.tensor_add(o[:], o[:], b1_sb[:])
                            neg = epool.tile([128, hc], f32, tag="neg")
                            nc.vector.tensor_scalar_min(neg[:], o[:], 0.0)
                            nc.scalar.activation(neg[:], neg[:], AF.Exp)
                            pos = epool.tile([128, hc], f32, tag="pos")
                            nc.vector.tensor_scalar_max(pos[:], o[:], 0.0)
                            act = epool.tile([128, hc], f16, tag="act")
                            # act = (exp(neg) - 1) + pos  (ELU), fused on DVE
                            nc.vector.scalar_tensor_tensor(
                                act[:], neg[:], -1.0, pos[:],
                                OP.add, OP.add)
                            if _epi == 7:
                                nc.vector.tensor_copy(x2T[:, w * 128:(w + 1) * 128], act[:])
                            else:
                                psT = psA.tile([128, 128], f16, tag="ps_node")
                                nc.tensor.transpose(psT[:], act[:], ident[:])
                                nc.vector.tensor_copy(x2T[:, w * 128:(w + 1) * 128], psT[:])
                        else:
                            if wi == 0:
                                opair = epool.tile([128, gw, C2], f32, tag="opair")
                            nc.vector.tensor_mul(
                                opair[:, wi, :].rearrange("p (h c) -> p h c", h=heads),
                                pw[wi][:, 0:hc].rearrange("p (h c) -> p h c", h=heads),
                                rec[:].broadcast_to([128, heads, ch]))
                            nc.vector.tensor_add(opair[:, wi, :], opair[:, wi, :], b2_sb[:])
                            if wi == gw - 1:
                                nc.sync.dma_start(
                                    out_d[w0 * 128:(w0 + gw) * 128, :]
                                    .rearrange("(i p) c -> p i c", p=128),
                                    opair[:])

            if KSTOP in (0, 3, 4):
                edge_phase(1)

            # ================= phase C: layer-2 node matmul =================
            for w in range(W if KSTOP in (0, 4) else 0):
                ps = psA.tile([128, 66], f32, tag="ps_node")
                nc.tensor.matmul(ps[:], lhsT=x2T[:, w * 128:(w + 1) * 128],
                                 rhs=w2e_f16[:], start=True, stop=True)
                if w % 2 == 0:
                    rows = rowpool.tile([128, 2, ROW2], f16, tag="rows2")
                rv = rows[:, w % 2, :]
                nc.vector.memset(rv[:, 66:], 0.0)
                nc.vector.tensor_copy(rv[:, 0:66], ps[:])
                nc.vector.tensor_copy(adsb2[:, w, :], ps[:, 65:66])
                if w % 2 == 1:
                    nc.sync.dma_start(
                        tab2_slice[(w - 1) * 128:(w + 1) * 128, :]
                        .rearrange("(i p) r -> p i r", p=128),
                        rows[:])
            if KSTOP in (0, 4):
                nc.gpsimd.collective_compute(
                    "AllGather", mybir.AluOpType.bypass,
                    replica_groups=[list(range(NCORES))],
                    ins=[tab2_slice.opt()], outs=[tab2_full.opt()],
                )

            if KSTOP == 0:
                edge_phase(2)

    nc.compile()
    return nc


# ---------------------------------------------------------------------------
# Entry point
# ---------------------------------------------------------------------------

_CACHE = {}


def _prepare(inputs):
    x = np.ascontiguousarray(np.asarray(inputs["x"], np.float32))
    ei = np.asarray(inputs["edge_index"])
    n_nodes = x.shape[0]
    return pick_config(x, ei, n_nodes)


def _weights_ext(inputs):
    W1 = np.asarray(inputs["W1"], np.float32)
    as1 = np.asarray(inputs["att_src1"], np.float32)
    ad1 = np.asarray(inputs["att_dst1"], np.float32)
    W2 = np.asarray(inputs["W2"], np.float32)
    as2 = np.asarray(inputs["att_src2"], np.float32)
    ad2 = np.asarray(inputs["att_dst2"], np.float32)
    As = np.zeros((HC1, H1), np.float32)
    Ad = np.zeros((HC1, H1), np.float32)
    for h in range(H1):
        As[h * C1:(h + 1) * C1, h] = as1[0, h]
        Ad[h * C1:(h + 1) * C1, h] = ad1[0, h]
    w1e = np.concatenate([W1, W1 @ As, W1 @ Ad], axis=1)           # [128,136]
    w2e = np.concatenate([W2, W2 @ as2[0].T, W2 @ ad2[0].T], axis=1)  # [128,66]
    return np.ascontiguousarray(w1e).astype(np.float16), np.ascontiguousarray(w2e)


def kernel(**inputs):
    from concourse.bass_utils import run_bass_kernel_spmd

    prep = _prepare(inputs)
    key = (prep["W"], prep["K"], prep["k_lo"], prep["k_hi"], prep["gw"], prep["hi_exists"])
    if key not in _CACHE:
        _CACHE[key] = build_program(dict(
            W=prep["W"], P=prep["P"], K=prep["K"], k_lo=prep["k_lo"],
            k_hi=prep["k_hi"], gw=prep["gw"], hi_exists=prep["hi_exists"]))
    nc = _CACHE[key]

    in_maps = build_in_maps(inputs, prep)
    res = run_bass_kernel_spmd(nc, in_maps, core_ids=list(range(NCORES)))
    return assemble_output(res.results, prep)


def build_in_maps(inputs, prep):
    x = np.ascontiguousarray(np.asarray(inputs["x"], np.float32))
    b1 = np.tile(np.asarray(inputs["b1"], np.float32).reshape(1, HC1), (128, 1))
    b2 = np.tile(np.asarray(inputs["b2"], np.float32).reshape(1, C2), (128, 1))
    w1e, w2e = _weights_ext(inputs)
    n_nodes, P, W = prep["n_nodes"], prep["P"], prep["W"]
    pi = prep["pi"]
    iota_r = np.tile(np.arange(128, dtype=np.float16), (128, 1))
    ident = np.eye(128, dtype=np.float16)
    in_maps = []
    # xT_pad per core: columns = padded slots
    xT_all = np.zeros((NCORES, IN_CH, P), np.float16)
    node_ids = np.arange(n_nodes)
    c_of = pi // P
    col = pi % P
    xT_all[c_of, :, col] = x[node_ids]  # fancy: sets [ch] vectors
    for c in range(NCORES):
        m = prep["meta"][c]
        im = dict(
            xT=np.ascontiguousarray(xT_all[c]),
            w1e=w1e, w2e=w2e,
            idx_lo=np.ascontiguousarray(m["idx_lo"]),
            dc=np.ascontiguousarray(m["dc"]),
            smh=np.ascontiguousarray(m["sm"]),
            iota_r=iota_r, ident=ident,
            b1=b1, b2=b2,
        )
        if prep["hi_exists"]:
            im["idx_hi"] = np.ascontiguousarray(m["idx_hi"])
        in_maps.append(im)
    return in_maps


def assemble_output(results, prep):
    P, n_nodes = prep["P"], prep["n_nodes"]
    full = np.concatenate([results[c]["out"] for c in range(NCORES)], axis=0)
    return np.ascontiguousarray(full[prep["pi"]]).astype(np.float32)



# revision 61
# speedup vs baseline: 60.0253x; 3.4864x over previous
"""Trainium2 Bass kernel for a 2-layer GAT encoder (edge-softmax message passing).

Strategy (8 NeuronCores, SPMD single program):
- dst-node partition across cores; host packs each core's dst nodes into
  fixed-count "windows" (<=128 nodes each) and edges into fixed-count
  128-edge tiles per window (K_LO tiles for src in the low half of the
  padded node space, K_HI for the high half -- dma_gather indices are int16).
- Node phase: h_ext = x @ [W | W@att_src | W@att_dst] per core slice
  (node-major matmuls with x^T chunks stationary), fp16 row table written to
  DRAM, AllGather -> full table on every core.
- Edge phase per 128-edge tile: dma_gather rows by src; the dst one-hot e1
  is built on DVE by an iota-vs-dstrel compare (packed fp16 operands for 2x
  DVE mode, dst-slot column pre-broadcast on the idle Act engine); its
  transpose sm is shipped precomputed from the host (same DMA bytes as
  shipping the slot ids, zero DVE build cost); per-edge a_d via sm matmul;
  p = exp(leaky_relu(a_s+a_d)) (softmax shift invariance makes the
  segment-max subtraction unnecessary); segment-sum of [p*h | p] via one-hot
  matmul accumulated in PSUM per window; epilogue divides and applies
  bias/ELU (ELU tail fused via scalar_tensor_tensor).
- Node-phase matmuls run in fp16 (x and W1 shipped as fp16); x chunks are
  loaded 4 windows per DMA and table rows written 2 windows per DMA to
  relieve the HWDGE dispatch queue; L2 outputs written 2 windows per DMA.
- sm is stored/loaded as fp8 (one-hot values are exact in fp8) with the a_d
  table also fp8 for matmul dtype match (quantizes a_d; rel err 2.6e-3, well
  under the 2e-2 gate); the SBUF this frees holds a per-group Act-engine psc
  expansion so the R-build is one packed-fp16 2x DVE multiply.
- sm/idx tiles use deep pools so their loads prefetch during the AllGather;
  3-deep gather pool pipelines groups across the edge phase.
- Output rows are window-padded; host de-permutes to the original node order.
"""

import numpy as np
from ml_dtypes import float8_e4m3fn as _f8

NCORES = 8
HALF = 32768          # int16 gather index limit
ROW1 = 256            # fp16 elems per layer-1 table row (512B): h128|a_s4|a_d4|pad
ROW2 = 128            # fp16 elems per layer-2 table row (256B): h64|a_s|a_d|pad
H1, C1 = 4, 32
H2, C2 = 1, 64
IN_CH = 128
HC1 = H1 * C1         # 128
NEG_SLOPE = 0.2
EPS = 1e-16


# ---------------------------------------------------------------------------
# Host-side preprocessing
# ---------------------------------------------------------------------------

def _pack_windows(src, dst, n_nodes, k_lo, k_hi, boundary):
    """Greedy-pack each core's dst nodes into windows (<=128 nodes, <=k_lo
    lo-tiles, <=k_hi hi-tiles). Returns per-core list of windows; each window
    is (node_lo, node_hi, lo_edge_srcs, hi_edge_srcs, lo_dstrel, hi_dstrel).
    Edges must be sorted by dst."""
    per_core = n_nodes // NCORES
    cores = []
    # bucket edges by dst once
    order = np.argsort(dst, kind="stable")
    src_s, dst_s = src[order], dst[order]
    # node -> edge range (dst-sorted)
    counts = np.bincount(dst_s, minlength=n_nodes)
    starts = np.concatenate([[0], np.cumsum(counts)])
    for c in range(NCORES):
        lo_n, hi_n = c * per_core, (c + 1) * per_core
        wins = []
        n = lo_n
        while n < hi_n:
            w_nodes = 0
            w_lo = []
            w_hi = []
            w_lo_dr = []
            w_hi_dr = []
            base = n
            while n < hi_n and w_nodes < 128:
                e0, e1 = starts[n], starts[n + 1]
                es = src_s[e0:e1]
                lo_m = es < boundary
                nlo = int(lo_m.sum())
                nhi = es.shape[0] - nlo
                cur_lo = sum(len(a) for a in w_lo)
                cur_hi = sum(len(a) for a in w_hi)
                if cur_lo + nlo > k_lo * 128 or cur_hi + nhi > k_hi * 128:
                    break
                w_lo.append(es[lo_m])
                w_hi.append(es[~lo_m])
                w_lo_dr.append(np.full(nlo, w_nodes, np.int32))
                w_hi_dr.append(np.full(nhi, w_nodes, np.int32))
                w_nodes += 1
                n += 1
            assert w_nodes > 0, "single node exceeds tile budget"
            wins.append((base, n,
                         np.concatenate(w_lo) if w_lo else np.zeros(0, src.dtype),
                         np.concatenate(w_hi) if w_hi else np.zeros(0, src.dtype),
                         np.concatenate(w_lo_dr) if w_lo_dr else np.zeros(0, np.int32),
                         np.concatenate(w_hi_dr) if w_hi_dr else np.zeros(0, np.int32)))
        cores.append(wins)
    return cores


def host_prep(x, edge_index, n_nodes, k_lo, k_hi, gw):
    """Build the permutation, per-core metadata and index arrays."""
    src = np.ascontiguousarray(edge_index[0]).astype(np.int64)
    dst = np.ascontiguousarray(edge_index[1]).astype(np.int64)
    per_core = n_nodes // NCORES
    assert per_core * NCORES == n_nodes

    # fixpoint on the lo/hi boundary in *real node id* space (pi is monotone)
    boundary = min(n_nodes, HALF)
    for _ in range(6):
        cores = _pack_windows(src, dst, n_nodes, k_lo, k_hi, boundary)
        W = max(len(w) for w in cores)
        W = ((W + gw - 1) // gw) * gw  # pad to group multiple
        P = W * 128
        # pi: node -> padded slot id
        pi = np.zeros(n_nodes, np.int64)
        for c, wins in enumerate(cores):
            for w, (a, b, *_r) in enumerate(wins):
                ids = np.arange(a, b)
                pi[ids] = c * P + w * 128 + (ids - a)
        nb = int(np.searchsorted(pi, HALF))  # first node with pi >= HALF
        if nb == boundary or P * NCORES <= HALF:
            boundary = nb if P * NCORES > HALF else n_nodes
            break
        boundary = nb
    cores = _pack_windows(src, dst, n_nodes, k_lo, k_hi, boundary)
    W = max(len(w) for w in cores)
    W = ((W + gw - 1) // gw) * gw
    P = W * 128
    pi = np.zeros(n_nodes, np.int64)
    for c, wins in enumerate(cores):
        for w, (a, b, *_r) in enumerate(wins):
            ids = np.arange(a, b)
            pi[ids] = c * P + w * 128 + (ids - a)
    assert P * NCORES <= 65536, f"padded node space {P*NCORES} exceeds uint16 gather range"
    hi_exists = P * NCORES > HALF
    if not hi_exists:
        assert all(len(w[3]) == 0 for ws in cores for w in ws)

    K = k_lo + k_hi
    meta = []
    for c, wins in enumerate(cores):
        idx_lo = np.zeros((W, k_lo * 128), np.int16)
        idx_hi = np.zeros((W, k_hi * 128), np.int16)
        drel = np.full((W, K, 128), 255, np.int32)  # [window, tile-in-window, slot]
        for w, (a, b, lo_s, hi_s, lo_dr, hi_dr) in enumerate(wins):
            pl = pi[lo_s]
            assert (pl < HALF).all()
            idx_lo[w, :len(pl)] = pl.astype(np.int16)
            # lo tiles occupy tile slots [0, k_lo)
            dr_pad = np.full(k_lo * 128, 255, np.int32)
            dr_pad[:len(lo_dr)] = lo_dr
            drel[w, :k_lo] = dr_pad.reshape(k_lo, 128)
            if hi_exists:
                ph = pi[hi_s] - HALF
                assert (ph >= 0).all() and (ph < 32768).all()
                idx_hi[w, :len(ph)] = ph.astype(np.int16)
            dr_pad = np.full(k_hi * 128, 255, np.int32)
            dr_pad[:len(hi_dr)] = hi_dr
            drel[w, k_lo:] = dr_pad.reshape(k_hi, 128) if k_hi else drel[w, k_lo:]
        # group-tile order: per group: [lo tiles of gw windows][hi tiles of gw windows]
        G = W // gw
        tile_order = []  # (window, tile-in-window-index)
        for g in range(G):
            for w in range(g * gw, (g + 1) * gw):
                tile_order += [(w, t) for t in range(k_lo)]
            for w in range(g * gw, (g + 1) * gw):
                tile_order += [(w, k_lo + t) for t in range(k_hi)]
        to = np.array(tile_order)
        drel_t = drel[to[:, 0], to[:, 1]]            # [W*K, 128]
        # idx arrays in 16-partition wrapped layout: idx j -> [j%16, j//16]
        def wrap16(a):
            a = a.reshape(-1)
            # idx j lives at [j%16, j//16], replicated into all 8 Q7 core
            # partition groups (HW reads each group independently)
            return np.ascontiguousarray(np.tile(a.reshape(-1, 16).T, (8, 1)))
        meta.append(dict(
            idx_lo=wrap16(idx_lo),
            idx_hi=wrap16(idx_hi) if hi_exists else None,
            dc=np.ascontiguousarray(drel_t.T).astype(np.float16),   # [128, W*K]
            # host-computed sm one-hot: sm[p, t*128+j] = (p == drel_t[t, j]);
            # same DMA bytes as shipping the slot ids broadcast, zero DVE build
            sm=np.ascontiguousarray(
                (np.arange(128, dtype=np.int32)[:, None, None] == drel_t[None, :, :])
                .astype(_f8).reshape(128, -1)),                      # [128, W*K*128]
        ))
    return dict(cores=cores, pi=pi, W=W, P=P, K=K, k_lo=k_lo, k_hi=k_hi, gw=gw,
                hi_exists=hi_exists, meta=meta, n_nodes=n_nodes, per_core=per_core)


def pick_config(x, edge_index, n_nodes):
    """Try candidate (k_lo, k_hi) packings, return the prep with fewest tiles."""
    gw = 2
    E = edge_index.shape[1]
    lam = E / n_nodes * 128
    base_lo = max(int(np.ceil(lam * 0.64 / 128)), 1)
    base_hi = max(int(np.ceil(lam * 0.36 / 128)), 0)
    cands = []
    for dlo in (-1, 0, 1, 2):
        for dhi in (-1, 0, 1, 2):
            if base_lo + dlo >= 1 and base_hi + dhi >= 0:
                cands.append((base_lo + dlo, base_hi + dhi))
    cands.sort(key=lambda c: c[0] + c[1])
    best = None
    for k_lo, k_hi in cands:
        try:
            p = host_prep(x, edge_index, n_nodes, k_lo, k_hi, gw)
        except AssertionError:
            continue
        slots = p["W"] * p["K"]
        if best is None or slots < best["W"] * best["K"]:
            best = p
        if slots <= (E / NCORES) / 128 * 1.08:  # good enough
            break
    assert best is not None, "no feasible packing found"
    return best


# ---------------------------------------------------------------------------
# Bass program
# ---------------------------------------------------------------------------

def build_program(cfg):
    import os
    KSTOP = int(os.environ.get("KSTOP", "0"))  # debug: 1=phaseA 2=+AG1 3=+edge1 4=+phaseC+AG2
    import concourse.bacc as bacc
    import concourse.bass as bass
    import concourse.mybir as mybir
    from concourse import tile

    f32 = mybir.dt.float32
    f16 = mybir.dt.float16
    i16 = mybir.dt.int16
    AF = mybir.ActivationFunctionType
    OP = mybir.AluOpType

    W, P, K, k_lo, k_hi, gw = cfg["W"], cfg["P"], cfg["K"], cfg["k_lo"], cfg["k_hi"], cfg["gw"]
    hi_exists = cfg["hi_exists"]
    G = W // gw
    T = gw * K                  # tiles per group
    GCAP = 1024                 # max gather descriptors per SWDGE call
    NLO = gw * k_lo * 128       # lo gather idxs per group
    NHI = gw * k_hi * 128
    P_ALL = P * NCORES

    nc = bacc.Bacc("TRN2", target_bir_lowering=False, debug=False, num_devices=NCORES,
                   dynamic_dma_scratch_size=36864)

    # ---- external inputs ----
    xT = nc.dram_tensor("xT", [IN_CH, P], f16, kind="ExternalInput")
    w1e = nc.dram_tensor("w1e", [IN_CH, 136], f16, kind="ExternalInput")
    w2e = nc.dram_tensor("w2e", [HC1, 66], f32, kind="ExternalInput")
    idx_lo_d = nc.dram_tensor("idx_lo", [128, W * k_lo * 8], i16, kind="ExternalInput")
    if hi_exists:
        idx_hi_d = nc.dram_tensor("idx_hi", [128, W * k_hi * 8], i16, kind="ExternalInput")
    u8 = mybir.dt.uint8
    dc_d = nc.dram_tensor("dc", [128, W * K], f16, kind="ExternalInput")
    f8 = mybir.dt.float8e4
    smh_d = nc.dram_tensor("smh", [128, W * K * 128], f8, kind="ExternalInput")
    iota_r_d = nc.dram_tensor("iota_r", [128, 128], f16, kind="ExternalInput")
    ident_d = nc.dram_tensor("ident", [128, 128], f16, kind="ExternalInput")
    b1_d = nc.dram_tensor("b1", [128, HC1], f32, kind="ExternalInput")
    b2_d = nc.dram_tensor("b2", [128, C2], f32, kind="ExternalInput")
    out_d = nc.dram_tensor("out", [P, C2], f32, kind="ExternalOutput")

    with tile.TileContext(nc) as tc:
        with (
            tc.tile_pool(name="const", bufs=1) as cpool,
            tc.tile_pool(name="xc", bufs=3) as xcpool,
            tc.tile_pool(name="rows", bufs=3) as rowpool,
            tc.tile_pool(name="gather", bufs=3) as gpool,
            tc.tile_pool(name="onehot", bufs=2) as opool,
            tc.tile_pool(name="rmat", bufs=2) as rpool,
            tc.tile_pool(name="scal", bufs=3) as spool,
            tc.tile_pool(name="idx", bufs=6) as ipool,
            tc.tile_pool(name="smp", bufs=3) as smpool,
            tc.tile_pool(name="epi", bufs=3) as epool,
            tc.tile_pool(name="psA", bufs=2, space="PSUM") as psA,
            tc.tile_pool(name="psW", bufs=4, space="PSUM") as psW,
            tc.tile_pool(name="psad", bufs=2, space="PSUM") as psad,
            tc.tile_pool(name="dram", bufs=1, space="DRAM") as dpool,
        ):
            # ---- constants to SBUF ----
            w1e_sb = cpool.tile([IN_CH, 136], f16, tag="w1e")
            nc.sync.dma_start(w1e_sb[:], w1e[:])
            w2e_sb = cpool.tile([HC1, 66], f32, tag="w2e")
            nc.sync.dma_start(w2e_sb[:], w2e[:])
            w2e_f16 = cpool.tile([HC1, 66], f16, tag="w2e16")
            nc.vector.tensor_copy(w2e_f16[:], w2e_sb[:])
            iota_r = cpool.tile([128, 128], f16, tag="iota_r")
            nc.sync.dma_start(iota_r[:], iota_r_d[:])
            ident = cpool.tile([128, 128], f16, tag="ident")
            nc.sync.dma_start(ident[:], ident_d[:])
            b1_sb = cpool.tile([128, HC1], f32, tag="b1")
            nc.sync.dma_start(b1_sb[:], b1_d[:])
            b2_sb = cpool.tile([128, C2], f32, tag="b2")
            nc.sync.dma_start(b2_sb[:], b2_d[:])
            adsb1 = cpool.tile([128, W, H1], f8, tag="adsb1")
            adsb2 = cpool.tile([128, W, H2], f8, tag="adsb2")
            x2T = cpool.tile([128, P], f16, tag="x2T")

            # ---- DRAM tables ----
            tab1_slice = dpool.tile([P, ROW1], f16, tag="t1s")
            tab1_full = dpool.tile([P_ALL, ROW1], f16, tag="t1f", addr_space="Shared")
            tab2_slice = dpool.tile([P, ROW2], f16, tag="t2s")
            tab2_full = dpool.tile([P_ALL, ROW2], f16, tag="t2f", addr_space="Shared")

            # ================= phase A: layer-1 node matmul =================
            import os as _os2
            _kpha = _os2.environ.get("KPHA") == "1"  # debug: no matmuls in phase A
            XCHUNK = 4
            for w in range(W):
                if w % XCHUNK == 0:
                    nw = min(XCHUNK, W - w)
                    xc = xcpool.tile([IN_CH, XCHUNK * 128], f16, tag="xc")
                    nc.sync.dma_start(xc[:, 0:nw * 128],
                                      xT[:, w * 128:(w + nw) * 128])
                xcv = xc[:, (w % XCHUNK) * 128:(w % XCHUNK + 1) * 128]
                if w % 2 == 0:
                    rows = rowpool.tile([128, 2, ROW1], f16, tag="rows1")
                rv = rows[:, w % 2, :]
                if _kpha:
                    nc.vector.memset(rv, 0.25)
                    nc.vector.memset(adsb1[:, w, :], 0.25)
                else:
                    ps = psA.tile([128, 136], f32, tag="ps_node")
                    nc.tensor.matmul(ps[:], lhsT=xcv, rhs=w1e_sb[:], start=True, stop=True)
                    nc.vector.memset(rv[:, 136:], 0.0)
                    nc.vector.tensor_copy(rv[:, 0:136], ps[:])
                    nc.vector.tensor_copy(adsb1[:, w, :], ps[:, 132:136])
                if w % 2 == 1:
                    nc.sync.dma_start(
                        tab1_slice[(w - 1) * 128:(w + 1) * 128, :]
                        .rearrange("(i p) r -> p i r", p=128),
                        rows[:])
            if KSTOP != 1:
                nc.gpsimd.collective_compute(
                    "AllGather", mybir.AluOpType.bypass,
                    replica_groups=[list(range(NCORES))],
                    ins=[tab1_slice.opt()], outs=[tab1_full.opt()],
                )

            # ================= edge phase (shared for both layers) ==========
            def edge_phase(layer):
                import os as _os
                _ked = int(_os.environ.get("KEDGE", "9"))
                _nog = _os.environ.get("KNOGATHER") == "1"

                if layer == 1:
                    table, row, heads, ch = tab1_full, ROW1, H1, C1
                    adsb = adsb1
                    a_s_off, a_d_off = 128, 132
                    rcols = HC1 + H1  # 132
                else:
                    table, row, heads, ch = tab2_full, ROW2, H2, C2
                    adsb = adsb2
                    a_s_off, a_d_off = 64, 65
                    rcols = C2 + H2  # 65
                hc = heads * ch
                nh = heads  # scalar cols per tile
                for g in range(G):
                    w0 = g * gw
                    # --- gather ---
                    Gt = gpool.tile([128, T, row], f16, tag=f"G{layer}")
                    il = ipool.tile([128, NLO // 16], i16, tag="il")
                    nc.sync.dma_start(il[:], idx_lo_d[:, g * (NLO // 16):(g + 1) * (NLO // 16)])
                    if not _nog:
                        for off in range(0, NLO, GCAP):
                            sz = min(GCAP, NLO - off)
                            nc.gpsimd.dma_gather(
                                out_ap=Gt[:, off // 128:(off + sz) // 128, :],
                                in_ap=table[0:min(HALF, P_ALL), :],
                                idxs_ap=il[:, off // 16:(off + sz) // 16],
                                num_idxs=sz, num_idxs_reg=sz,
                                elem_size=row)
                    else:
                        nc.vector.memset(Gt[:].rearrange("p t r -> p (t r)"), 0.25)
                    if hi_exists and k_hi > 0:
                        ih = ipool.tile([128, NHI // 16], i16, tag="ih")
                        nc.sync.dma_start(ih[:], idx_hi_d[:, g * (NHI // 16):(g + 1) * (NHI // 16)])
                        if not _nog:
                            for off in range(0, NHI, GCAP):
                                sz = min(GCAP, NHI - off)
                                nc.gpsimd.dma_gather(
                                    out_ap=Gt[:, gw * k_lo + off // 128:gw * k_lo + (off + sz) // 128, :],
                                    in_ap=table[HALF:P_ALL, :],
                                    idxs_ap=ih[:, off // 16:(off + sz) // 16],
                                    num_idxs=sz, num_idxs_reg=sz,
                                    elem_size=row)
                    # --- one-hot builds ---
                    if _ked < 2:
                        dmy = spool.tile([128, T, row], f16, tag="dmyG")
                        nc.vector.tensor_copy(dmy[:], Gt[:])
                        continue
                    dc_sb = ipool.tile([128, T], f16, tag="dc")
                    nc.sync.dma_start(dc_sb[:], dc_d[:, g * T:(g + 1) * T])
                    sm = smpool.tile([128, T, 128], f8, tag="sm")
                    nc.sync.dma_start(
                        sm[:].rearrange("p t j -> p (t j)"),
                        smh_d[:, g * T * 128:(g + 1) * T * 128])
                    # expand dc on the (idle) Act engine so the DVE compare
                    # sees packed 2-byte operands and runs in 2x mode
                    dc_exp = opool.tile([128, T, 128], f16, tag="dc_exp")
                    nc.scalar.activation(
                        dc_exp[:],
                        dc_sb[:].rearrange("p (t one) -> p t one", one=1).broadcast_to([128, T, 128]),
                        AF.Copy)
                    e1 = opool.tile([128, T, 128], f16, tag="e1")
                    nc.vector.tensor_tensor(
                        e1[:],
                        dc_exp[:],
                        iota_r[:].rearrange("p (one x) -> p one x", one=1).broadcast_to([128, T, 128]),
                        OP.is_equal)
                    if _ked < 3:
                        continue
                    # --- a_d broadcast matmuls ---
                    pad = psad.tile([128, T * nh], f32, tag="pad")
                    for t in range(T):
                        w = w0 + (t // k_lo if t < gw * k_lo else (t - gw * k_lo) // k_hi)
                        nc.tensor.matmul(pad[:, t * nh:(t + 1) * nh],
                                         lhsT=sm[:, t, :], rhs=adsb[:, w, :],
                                         start=True, stop=True)
                    if _ked < 4:
                        continue
                    # --- per-edge scalars ---
                    z = spool.tile([128, T, nh], f32, tag="z")
                    nc.vector.tensor_add(z[:], Gt[:, :, a_s_off:a_s_off + nh],
                                         pad[:].rearrange("p (t h) -> p t h", h=nh))
                    z2 = spool.tile([128, T, nh], f32, tag="z2")
                    nc.vector.tensor_scalar_mul(z2[:], z[:], NEG_SLOPE)
                    nc.vector.tensor_max(z[:], z[:], z2[:])
                    psc = spool.tile([128, T, nh], f16, tag="psc")
                    nc.scalar.activation(psc[:], z[:], AF.Exp)
                    if _ked < 5:
                        continue
                    # --- R = [p*h | p] ---
                    # expand psc per-head on the Act engine so the DVE mul is
                    # a single packed-f16 2x-mode op
                    pscx_flat = rpool.tile([128, T * 128], f16, tag="pscx")
                    pscx = pscx_flat[:, 0:T * hc].rearrange(
                        "p (t h c) -> p t h c", t=T, h=heads, c=ch)
                    nc.scalar.activation(
                        pscx,
                        psc[:].rearrange("p t (h one) -> p t h one", one=1)
                              .broadcast_to([128, T, heads, ch]),
                        AF.Copy)
                    R = rpool.tile([128, T, rcols], f16, tag=f"R")
                    nc.vector.tensor_mul(
                        R[:, :, 0:hc].rearrange("p t (h c) -> p t h c", h=heads),
                        Gt[:, :, 0:hc].rearrange("p t (h c) -> p t h c", h=heads),
                        pscx)
                    nc.vector.tensor_copy(R[:, :, hc:hc + nh], psc[:])
                    nc.vector.tensor_copy(R[:, :, hc:hc + nh], psc[:])
                    if _ked < 6:
                        continue
                    # --- segment-sum matmuls ---
                    pw = [psW.tile([128, rcols], f32, tag="psW", name=f"pw{g}_{wi}")
                          for wi in range(gw)]
                    for t in range(T):
                        if t < gw * k_lo:
                            wi, first = divmod(t, k_lo)
                            is_first = first == 0
                            is_last = (first == k_lo - 1) and k_hi == 0
                        else:
                            wi, r = divmod(t - gw * k_lo, k_hi)
                            is_first = False
                            is_last = r == k_hi - 1
                        nc.tensor.matmul(pw[wi][:], lhsT=e1[:, t, :], rhs=R[:, t, :],
                                         start=is_first, stop=is_last)
                    # --- epilogue per window ---
                    import os as _os
                    _epi = int(_os.environ.get("KEPI", "0"))  # 6=no epilogue, 7=no transpose
                    for wi in range(gw):
                        if _epi == 6:
                            dummy = epool.tile([128, rcols], f16, tag="dummy")
                            nc.vector.tensor_copy(dummy[:], pw[wi][:])
                            continue
                        w = w0 + wi
                        den = epool.tile([128, nh], f32, tag="den")
                        nc.vector.tensor_scalar_add(den[:], pw[wi][:, hc:hc + nh], EPS)
                        rec = epool.tile([128, nh], f32, tag="rec")
                        nc.vector.reciprocal(rec[:], den[:])
                        if layer == 1:
                            o = epool.tile([128, hc], f32, tag="o")
                            nc.vector.tensor_mul(
                                o[:].rearrange("p (h c) -> p h c", h=heads),
                                pw[wi][:, 0:hc].rearrange("p (h c) -> p h c", h=heads),
                                rec[:].broadcast_to([128, heads, ch]))
                            nc.vector.tensor_add(o[:], o[:], b1_sb[:])
                            neg = epool.tile([128, hc], f32, tag="neg")
                            nc.vector.tensor_scalar_min(neg[:], o[:], 0.0)
                            nc.scalar.activation(neg[:], neg[:], AF.Exp)
                            pos = epool.tile([128, hc], f32, tag="pos")
                            nc.vector.tensor_scalar_max(pos[:], o[:], 0.0)
                            act = epool.tile([128, hc], f16, tag="act")
                            # act = (exp(neg) - 1) + pos  (ELU), fused on DVE
                            nc.vector.scalar_tensor_tensor(
                                act[:], neg[:], -1.0, pos[:],
                                OP.add, OP.add)
                            if _epi == 7:
                                nc.vector.tensor_copy(x2T[:, w * 128:(w + 1) * 128], act[:])
                            else:
                                psT = psA.tile([128, 128], f16, tag="ps_node")
                                nc.tensor.transpose(psT[:], act[:], ident[:])
                                nc.vector.tensor_copy(x2T[:, w * 128:(w + 1) * 128], psT[:])
                        else:
                            if wi == 0:
                                opair = epool.tile([128, gw, C2], f32, tag="opair")
                            nc.vector.tensor_mul(
                                opair[:, wi, :].rearrange("p (h c) -> p h c", h=heads),
                                pw[wi][:, 0:hc].rearrange("p (h c) -> p h c", h=heads),
                                rec[:].broadcast_to([128, heads, ch]))
                            nc.vector.tensor_add(opair[:, wi, :], opair[:, wi, :], b2_sb[:])
                            if wi == gw - 1:
                                nc.sync.dma_start(
                                    out_d[w0 * 128:(w0 + gw) * 128, :]
                                    .rearrange("(i p) c -> p i c", p=128),
                                    opair[:])

            if KSTOP in (0, 3, 4):
                edge_phase(1)

            # ================= phase C: layer-2 node matmul =================
            for w in range(W if KSTOP in (0, 4) else 0):
                ps = psA.tile([128, 66], f32, tag="ps_node")
                nc.tensor.matmul(ps[:], lhsT=x2T[:, w * 128:(w + 1) * 128],
                                 rhs=w2e_f16[:], start=True, stop=True)
                if w % 2 == 0:
                    rows = rowpool.tile([128, 2, ROW2], f16, tag="rows2")
                rv = rows[:, w % 2, :]
                nc.vector.memset(rv[:, 66:], 0.0)
                nc.vector.tensor_copy(rv[:, 0:66], ps[:])
                nc.vector.tensor_copy(adsb2[:, w, :], ps[:, 65:66])
                if w % 2 == 1:
                    nc.sync.dma_start(
                        tab2_slice[(w - 1) * 128:(w + 1) * 128, :]
                        .rearrange("(i p) r -> p i r", p=128),
                        rows[:])
            if KSTOP in (0, 4):
                nc.gpsimd.collective_compute(
                    "AllGather", mybir.AluOpType.bypass,
                    replica_groups=[list(range(NCORES))],
                    ins=[tab2_slice.opt()], outs=[tab2_full.opt()],
                )

            if KSTOP == 0:
                edge_phase(2)

    nc.compile()
    return nc


# ---------------------------------------------------------------------------
# Entry point
# ---------------------------------------------------------------------------

_CACHE = {}


def _prepare(inputs):
    x = np.ascontiguousarray(np.asarray(inputs["x"], np.float32))
    ei = np.asarray(inputs["edge_index"])
    n_nodes = x.shape[0]
    return pick_config(x, ei, n_nodes)


def _weights_ext(inputs):
    W1 = np.asarray(inputs["W1"], np.float32)
    as1 = np.asarray(inputs["att_src1"], np.float32)
    ad1 = np.asarray(inputs["att_dst1"], np.float32)
    W2 = np.asarray(inputs["W2"], np.float32)
    as2 = np.asarray(inputs["att_src2"], np.float32)
    ad2 = np.asarray(inputs["att_dst2"], np.float32)
    As = np.zeros((HC1, H1), np.float32)
    Ad = np.zeros((HC1, H1), np.float32)
    for h in range(H1):
        As[h * C1:(h + 1) * C1, h] = as1[0, h]
        Ad[h * C1:(h + 1) * C1, h] = ad1[0, h]
    w1e = np.concatenate([W1, W1 @ As, W1 @ Ad], axis=1)           # [128,136]
    w2e = np.concatenate([W2, W2 @ as2[0].T, W2 @ ad2[0].T], axis=1)  # [128,66]
    return np.ascontiguousarray(w1e).astype(np.float16), np.ascontiguousarray(w2e)


def kernel(**inputs):
    from concourse.bass_utils import run_bass_kernel_spmd

    prep = _prepare(inputs)
    key = (prep["W"], prep["K"], prep["k_lo"], prep["k_hi"], prep["gw"], prep["hi_exists"])
    if key not in _CACHE:
        _CACHE[key] = build_program(dict(
            W=prep["W"], P=prep["P"], K=prep["K"], k_lo=prep["k_lo"],
            k_hi=prep["k_hi"], gw=prep["gw"], hi_exists=prep["hi_exists"]))
    nc = _CACHE[key]

    in_maps = build_in_maps(inputs, prep)
    res = run_bass_kernel_spmd(nc, in_maps, core_ids=list(range(NCORES)))
    return assemble_output(res.results, prep)


def build_in_maps(inputs, prep):
    x = np.ascontiguousarray(np.asarray(inputs["x"], np.float32))
    b1 = np.tile(np.asarray(inputs["b1"], np.float32).reshape(1, HC1), (128, 1))
    b2 = np.tile(np.asarray(inputs["b2"], np.float32).reshape(1, C2), (128, 1))
    w1e, w2e = _weights_ext(inputs)
    n_nodes, P, W = prep["n_nodes"], prep["P"], prep["W"]
    pi = prep["pi"]
    iota_r = np.tile(np.arange(128, dtype=np.float16), (128, 1))
    ident = np.eye(128, dtype=np.float16)
    in_maps = []
    # xT_pad per core: columns = padded slots
    xT_all = np.zeros((NCORES, IN_CH, P), np.float16)
    node_ids = np.arange(n_nodes)
    c_of = pi // P
    col = pi % P
    xT_all[c_of, :, col] = x[node_ids]  # fancy: sets [ch] vectors
    for c in range(NCORES):
        m = prep["meta"][c]
        im = dict(
            xT=np.ascontiguousarray(xT_all[c]),
            w1e=w1e, w2e=w2e,
            idx_lo=np.ascontiguousarray(m["idx_lo"]),
            dc=np.ascontiguousarray(m["dc"]),
            smh=np.ascontiguousarray(m["sm"]),
            iota_r=iota_r, ident=ident,
            b1=b1, b2=b2,
        )
        if prep["hi_exists"]:
            im["idx_hi"] = np.ascontiguousarray(m["idx_hi"])
        in_maps.append(im)
    return in_maps


def assemble_output(results, prep):
    P, n_nodes = prep["P"], prep["n_nodes"]
    full = np.concatenate([results[c]["out"] for c in range(NCORES)], axis=0)
    return np.ascontiguousarray(full[prep["pi"]]).astype(np.float32)

